# revision 1
# baseline (speedup 1.0000x reference)
"""Trainium2 Bass kernel for nn_FactorCovModel.

Model: 2-layer LSTM (H=512) over [B=256, T=64, D=500], last hidden ->
FC [512 -> 16532] -> Sigma = Lambda diag(exp(fv)) Lambda^T + diag(exp(idio)),
output [256, 500, 500].

Sharding: pure data parallel over batch, 32 samples/core on 8 cores.

Per-core device algorithm (matmul operands bf16, fp32 PSUM accumulation):
  - Weight gate axis host-permuted so PSUM col group hg holds hidden slice
    hg of ALL FOUR gates: PSUM [128 = (hg, batch), 512 = i|f|g|o x 128].
    Every ACT/DVE op is then full-128-partition and partition-aligned.
  - LSTM gates are computed column-tiled: stationary = hT chunk [128, 32],
    4 hidden-slice groups run concurrently at tile positions (0, 32j).
  - x-projection (xg0) matmuls accumulate into the same PSUM tile two
    steps ahead; recurrent matmuls then add onto it (start=False).
  - Layer-1 consumes h0T[t] directly (fused input projection, contraction
    [h0T; h1T] = 1024) plus a bias inject via a stacked-identity stationary.
  - FC runs col-packed (4 feature tiles of 512 per PSUM tile), then Lambda
    is re-laid-out via 500 PE transposes of [32, 32] blocks into
    LT [32 factors, 500 assets, 32 batch]; fvar gets exp via ACT.
  - Sigma_b = (LT_b * f_b)^T @ LT_b per sample, 4 m-tiles of 128.
  - idio raw rows go back to the host, which applies bias+exp and adds the
    diagonal (avoids diagonal APs on device).
"""

import os
import sys

sys.path.insert(0, "/opt/trn_rl_repo")

import numpy as np

import concourse.bass as bass
import concourse.mybir as mybir
from concourse import bacc
from concourse.tile import TileContext

FP = mybir.dt.float32
BF = mybir.dt.bfloat16
AF = mybir.ActivationFunctionType

B_FULL, T_FULL, D_IN, H = 256, 64, 500, 512
NCORES = 8
BL = B_FULL // NCORES            # 32 samples per core
NA, NF = 500, 32                 # assets, factors
OUT_DIM = NA * NF + NF + NA      # 16532
NTILE = 512                      # FC feature tile
N_FTILES = 33                    # ceil(16532/512) -> features padded to 16896
FH = N_FTILES * NTILE            # 16896
XCHUNK = 16                      # time steps per streamed xT chunk

# gate-axis permutation: new col (hg, gate, hl) = 512*hg + 128*gate + hl maps to
# old row gate*512 + 128*hg + hl (torch gate order [i, f, g, o]).  With this
# layout, PSUM col group hg holds ALL FOUR gates of hidden slice hg along the
# free dim, so every ACT/DVE op is full-128-partition and partition-aligned.
PERM = np.array([gate * 512 + 128 * hg + hl
                 for hg in range(4) for gate in range(4) for hl in range(128)])


# ---------------------------------------------------------------- host prep

def host_prep_shared(inputs):
    w_ih0 = np.asarray(inputs["w_ih0"])[PERM]
    w_hh0 = np.asarray(inputs["w_hh0"])[PERM]
    b0 = (np.asarray(inputs["b_ih0"]) + np.asarray(inputs["b_hh0"]))[PERM]
    w_ih1 = np.asarray(inputs["w_ih1"])[PERM]
    w_hh1 = np.asarray(inputs["w_hh1"])[PERM]
    b1 = (np.asarray(inputs["b_ih1"]) + np.asarray(inputs["b_hh1"]))[PERM]
    fc_w = np.asarray(inputs["fc_w"])
    fc_b = np.asarray(inputs["fc_b"])

    w0T = np.zeros((512, 2048), np.float32)
    w0T[:500] = w_ih0.T
    w0T[500] = b0
    wh0T = np.ascontiguousarray(w_hh0.T, dtype=np.float32)
    w1T = np.ascontiguousarray(np.concatenate([w_ih1.T, w_hh1.T]), dtype=np.float32)
    b1T = np.zeros((128, 512), np.float32)
    for j in range(4):
        b1T[32 * j:32 * (j + 1), :] = b1[512 * j:512 * (j + 1)][None, :]
    fcwT = np.zeros((512, FH), np.float32)
    fcwT[:, :OUT_DIM] = fc_w.T
    fcbT = np.zeros((32, 512), np.float32)
    fcbT[:, :500] = fc_b[:16000].reshape(500, 32).T
    fcbF = np.ascontiguousarray(fc_b[16000:16032].reshape(32, 1), dtype=np.float32)
    ident = np.ascontiguousarray(np.tile(np.eye(32, dtype=np.float32), (4, 1)))
    import ml_dtypes
    tobf = lambda a: np.ascontiguousarray(a, dtype=ml_dtypes.bfloat16)
    return dict(w0T=tobf(w0T), wh0T=tobf(wh0T), w1T=tobf(w1T), b1T=b1T,
                fcwT=tobf(fcwT), fcbT=fcbT, fcbF=fcbF, identt=ident)


def host_prep_x(x_core):
    """x_core [BL, T, 500] -> xT [512, T*BL], (t, b) free order, ones bias row."""
    T = x_core.shape[1]
    import ml_dtypes
    xT = np.zeros((512, T * BL), np.float32)
    xT[:500] = np.asarray(x_core, np.float32).transpose(2, 1, 0).reshape(500, T * BL)
    xT[500] = 1.0
    return np.ascontiguousarray(xT, dtype=ml_dtypes.bfloat16)


# ---------------------------------------------------------------- bass build

def build_nc(T=T_FULL):
    nc = bacc.Bacc("TRN2")

    xT_d = nc.dram_tensor("xT", [512, T * BL], BF, kind="ExternalInput")
    w0T_d = nc.dram_tensor("w0T", [512, 2048], BF, kind="ExternalInput")
    wh0T_d = nc.dram_tensor("wh0T", [512, 2048], BF, kind="ExternalInput")
    w1T_d = nc.dram_tensor("w1T", [1024, 2048], BF, kind="ExternalInput")
    b1T_d = nc.dram_tensor("b1T", [128, 512], FP, kind="ExternalInput")
    fcwT_d = nc.dram_tensor("fcwT", [512, FH], BF, kind="ExternalInput")
    fcbT_d = nc.dram_tensor("fcbT", [32, 512], FP, kind="ExternalInput")
    fcbF_d = nc.dram_tensor("fcbF", [32, 1], FP, kind="ExternalInput")
    identt_d = nc.dram_tensor("identt", [128, 32], FP, kind="ExternalInput")

    sigma_d = nc.dram_tensor("sigma", [BL, NA, NA], FP, kind="ExternalOutput")
    idio_d = nc.dram_tensor("idio_raw", [BL, NA], FP, kind="ExternalOutput")

    def mm(out, lhsT, rhs, tp, **kw):
        nc.tensor.matmul(out, lhsT, rhs,
                         tile_position=tp, skip_group_check=True, **kw)

    def tr(out, in_, identity, tp):
        nc.tensor.matmul(out, in_, identity, is_transpose=True,
                         tile_position=tp, skip_group_check=True)

    with TileContext(nc) as tc:
        with tc.tile_pool(name="persist", bufs=1) as persist:
            b1T_sb = persist.tile([128, 512], FP)
            nc.sync.dma_start(b1T_sb, b1T_d[:, :])
            identt_sb = persist.tile([128, 32], FP)
            nc.sync.dma_start(identt_sb, identt_d[:, :])
            fcbT_sb = persist.tile([32, 512], FP)
            nc.sync.dma_start(fcbT_sb, fcbT_d[:, :])
            fcbF_sb = persist.tile([32, 1], FP)
            nc.sync.dma_start(fcbF_sb, fcbF_d[:, :])
            hlast = persist.tile([128, 128], BF)  # final h1T, chunk-major cols

            # ---------------- phase 1: LSTM ----------------
            with (
                tc.tile_pool(name="wconst", bufs=1) as wconst,
                tc.tile_pool(name="xring", bufs=2) as xring,
                tc.tile_pool(name="state", bufs=2) as state,
                tc.tile_pool(name="work", bufs=2) as work,
                tc.tile_pool(name="pgates", bufs=8, space="PSUM") as pgates,
            ):
                w0T_sb = wconst.tile([128, 4, 2048], BF)
                nc.sync.dma_start(w0T_sb, w0T_d.rearrange("(ko p) g -> p ko g", p=128))
                wh0T_sb = wconst.tile([128, 4, 2048], BF)
                nc.sync.dma_start(wh0T_sb, wh0T_d.rearrange("(ko p) g -> p ko g", p=128))
                w1T_sb = wconst.tile([128, 8, 2048], BF)
                nc.sync.dma_start(w1T_sb, w1T_d.rearrange("(ko p) g -> p ko g", p=128))

                xch = min(XCHUNK, T)
                n_xchunks = (T + xch - 1) // xch
                x_tiles = {}

                def load_xchunk(ci):
                    if ci >= n_xchunks:
                        return
                    xt = xring.tile([128, 4, xch * BL], BF, tag="xchunk")
                    nc.sync.dma_start(
                        xt,
                        xT_d[:, ci * xch * BL:(ci + 1) * xch * BL]
                        .rearrange("(ko p) tb -> p ko tb", p=128),
                    )
                    x_tiles[ci] = xt

                load_xchunk(0)

                def gates_banks(nm):
                    # one PSUM bank per hidden-slice col group: concurrent
                    # col-tiled matmuls into the SAME bank corrupt on HW.
                    return [pgates.tile([128, 512], FP, tag="g", name=f"{nm}{j}")
                            for j in range(4)]

                def evac(pgs, dst, bias):
                    """Copy group j of pgs -> dst[32j:32j+32] (+bias), 2 ACT / 2 DVE."""
                    for j in range(4):
                        s = slice(32 * j, 32 * (j + 1))
                        if bias is None:
                            if j % 2 == 0:
                                nc.scalar.copy(dst[s, :], pgs[j][s, :])
                            else:
                                nc.vector.tensor_copy(dst[s, :], pgs[j][s, :])
                        else:
                            nc.vector.tensor_add(dst[s, :], pgs[j][s, :],
                                                 bias[s, :])
                    return dst

                def gate_nonlin(ga, cprev, cnew, tag):
                    """ga [128 = (hg, b), 512 = i|f|g|o x128] SBUF -> (H, cnew)."""
                    a = work.tile([128, 512], FP, tag=f"act_{tag}")
                    nc.scalar.activation(a[:, 0:256], ga[:, 0:256], AF.Sigmoid)
                    nc.scalar.activation(a[:, 256:384], ga[:, 256:384], AF.Tanh)
                    nc.scalar.activation(a[:, 384:512], ga[:, 384:512], AF.Sigmoid)
                    t1 = work.tile([128, 128], FP, tag=f"t1_{tag}")
                    nc.vector.tensor_mul(t1, a[:, 0:128], a[:, 256:384])
                    if cprev is None:
                        cn = t1  # c_prev == 0 at t == 0
                    else:
                        t2 = work.tile([128, 128], FP, tag=f"t2_{tag}")
                        nc.vector.tensor_mul(t2, a[:, 128:256], cprev)
                        cn = cnew
                        nc.vector.tensor_add(cn, t1, t2)
                    tcn = work.tile([128, 128], FP, tag=f"tc_{tag}")
                    nc.scalar.activation(tcn, cn, AF.Tanh)
                    hh = work.tile([128, 128], FP, tag=f"h_{tag}")
                    nc.vector.tensor_mul(hh, a[:, 384:512], tcn)
                    return hh, cn

                def transpose_h(hh, tag):
                    """hh [128=(hg,b),128] -> hT bf16 [128,128]; each 32-col
                    transpose gets its OWN psum bank (concurrent row-tiled
                    transposes into one bank corrupt on HW)."""
                    ht = state.tile([128, 128], BF, tag=f"ht_{tag}")
                    for k in range(4):
                        ptk = pgates.tile([128, 32], FP, tag="g", name=f"pt{k}")
                        tr(ptk, hh[32 * k:32 * (k + 1), :],
                           identt_sb[32 * k:32 * (k + 1), :], (32 * k, 0))
                        nc.vector.tensor_copy(ht[:, 32 * k:32 * (k + 1)], ptk)
                    return ht

                c0 = c1 = None
                h0T = h1T = None
                for t in range(T):
                    ci, tl = t // xch, t % xch
                    if tl == 0:
                        load_xchunk(ci + 1)
                    xt = x_tiles[ci]

                    pgs0 = gates_banks("g0_")
                    last0 = (t == 0)
                    for k in range(4):
                        lhsT = xt[:, k, tl * BL:(tl + 1) * BL]
                        for j in range(4):
                            mm(pgs0[j][32 * j:32 * (j + 1), :], lhsT,
                               w0T_sb[:, k, 512 * j:512 * (j + 1)],
                               tp=(0, 32 * j),
                               start=(k == 0), stop=(last0 and k == 3))
                    if t > 0:
                        for k in range(4):
                            lhsT = h0T[:, 32 * k:32 * (k + 1)]
                            for j in range(4):
                                mm(pgs0[j][32 * j:32 * (j + 1), :], lhsT,
                                   wh0T_sb[:, k, 512 * j:512 * (j + 1)],
                                   tp=(0, 32 * j),
                                   start=False, stop=(k == 3))
                    a0 = work.tile([128, 512], FP, tag="ga_l0")
                    evac(pgs0, a0, None)
                    c0n = None if c0 is None else state.tile([128, 128], FP, tag="c0")
                    h0, c0 = gate_nonlin(a0, c0, c0n, "l0")
                    h0T = transpose_h(h0, "l0")

                    pgs1 = gates_banks("g1_")
                    nk = 4 if t == 0 else 8
                    for k in range(nk):
                        srct = h0T if k < 4 else h1T
                        lhsT = srct[:, 32 * (k % 4):32 * (k % 4 + 1)]
                        for j in range(4):
                            mm(pgs1[j][32 * j:32 * (j + 1), :], lhsT,
                               w1T_sb[:, k, 512 * j:512 * (j + 1)],
                               tp=(0, 32 * j),
                               start=(k == 0), stop=(k == nk - 1))
                    a1 = work.tile([128, 512], FP, tag="ga_l1")
                    evac(pgs1, a1, b1T_sb)
                    c1n = None if c1 is None else state.tile([128, 128], FP, tag="c1")
                    h1, c1 = gate_nonlin(a1, c1, c1n, "l1")
                    h1T = transpose_h(h1, "l1")

                nc.vector.tensor_copy(hlast, h1T)

            # ---------------- phase 2: FC + Lambda layout + Sigma ----------------
            with (
                tc.tile_pool(name="fcw", bufs=3) as fcwp,
                tc.tile_pool(name="rawp", bufs=3) as rawp,
                tc.tile_pool(name="lt", bufs=1) as ltp,
                tc.tile_pool(name="sigw", bufs=4) as sigw,
                tc.tile_pool(name="pfc", bufs=4, space="PSUM") as pfcp,
                tc.tile_pool(name="plt", bufs=2, space="PSUM") as pltp,
                tc.tile_pool(name="psig", bufs=2, space="PSUM") as psigp,
            ):
                LT = ltp.tile([32, 500, 32], BF)       # [factor, asset, b]
                F_sb = ltp.tile([32, 32], FP)          # exp(fvar raw + bias) [factor, b]

                n_quads = (N_FTILES + 3) // 4          # 9 (last quad has 1 tile)
                for q in range(n_quads):
                    rr = range(4) if q < 8 else range(1)
                    raw_t = rawp.tile([128, 512], FP, tag="raw")
                    for r in rr:
                        jj = 4 * q + r
                        fcw_t = fcwp.tile([128, 4, 512], BF, tag="fcw")
                        nc.sync.dma_start(
                            fcw_t,
                            fcwT_d[:, jj * 512:(jj + 1) * 512]
                            .rearrange("(ko p) n -> p ko n", p=128),
                        )
                        # one PSUM bank per n-tile (col groups must not share)
                        pfc = pfcp.tile([128, 512], FP, tag="pfc")
                        for k in range(4):
                            mm(pfc[32 * r:32 * (r + 1), :],
                               hlast[:, 32 * k:32 * (k + 1)],
                               fcw_t[:, k, :],
                               tp=(0, 32 * r),
                               start=(k == 0), stop=(k == 3))
                        s = slice(32 * r, 32 * (r + 1))
                        if r % 2 == 0:
                            nc.scalar.copy(raw_t[s, :], pfc[s, :])
                        else:
                            nc.vector.tensor_copy(raw_t[s, :], pfc[s, :])

                    # Lambda blocks inside this quad -> transpose into LT
                    for r in rr:
                        jj = 4 * q + r
                        base_feat = jj * 512
                        nblk = 16 if jj < 31 else (4 if jj == 31 else 0)
                        for g in range(0, nblk, 4):
                            pt = pltp.tile([32, 128], FP, tag="plt")
                            for a in range(4):
                                blk = g + a
                                tr(pt[:, 32 * a:32 * (a + 1)],
                                   raw_t[32 * r:32 * (r + 1),
                                         32 * blk:32 * (blk + 1)],
                                   identt_sb[32 * r:32 * (r + 1), :], (32 * r, 0))
                            a0 = (base_feat + 32 * g) // 32  # first asset index
                            nc.vector.tensor_tensor(
                                LT[:, a0:a0 + 4, :],
                                pt.rearrange("f (a b) -> f a b", a=4),
                                fcbT_sb[:, a0:a0 + 4, None].to_broadcast([32, 4, 32]),
                                mybir.AluOpType.add,
                            )
                        if jj == 31:
                            # fvar: features 16000:16032 = cols 128:160 (r == 3)
                            ptf_full = pltp.tile([32, 128], FP, tag="plt")
                            ptf = ptf_full[:, 0:32]
                            tr(ptf, raw_t[96:128, 128:160],
                               identt_sb[96:128, :], (96, 0))
                            nc.scalar.activation(F_sb, ptf, AF.Exp,
                                                 bias=fcbF_sb[:, 0:1])
                            # idio part 1: features 16032:16384 = cols 160:512
                            nc.sync.dma_start(idio_d[:, 0:352],
                                              raw_t[96:128, 160:512])
                        if jj == 32:
                            # idio part 2: features 16384:16532 = cols 0:148
                            nc.sync.dma_start(idio_d[:, 352:500],
                                              raw_t[0:32, 0:148])

                # Sigma per sample
                for b in range(BL):
                    gt = sigw.tile([32, 512], BF, tag="gt")
                    nc.vector.tensor_scalar_mul(gt[:, 0:500], LT[:, :, b],
                                                F_sb[:, b:b + 1])
                    for mt in range(4):
                        rows = 128 if mt < 3 else 116
                        ps = psigp.tile([128, 512], FP, tag="psig")
                        mm(ps[:rows, 0:500], gt[:, 128 * mt:128 * mt + rows],
                           LT[:, :, b], tp=(0, 0), start=True, stop=True)
                        st = sigw.tile([128, 512], FP, tag="sigstage")
                        if mt % 2 == 0:
                            nc.scalar.copy(st[:rows, 0:500], ps[:rows, 0:500])
                        else:
                            nc.vector.tensor_copy(st[:rows, 0:500], ps[:rows, 0:500])
                        nc.sync.dma_start(
                            sigma_d[b, 128 * mt:128 * mt + rows, :],
                            st[:rows, 0:500])

    nc.compile()
    return nc


# ---------------------------------------------------------------- entry point

def kernel(**inputs):
    from concourse.bass_utils import run_bass_kernel_spmd

    prep = host_prep_shared(inputs)
    x = np.asarray(inputs["x"], np.float32)
    in_maps = []
    for core in range(NCORES):
        m = dict(prep)
        m["xT"] = host_prep_x(x[core * BL:(core + 1) * BL])
        in_maps.append(m)

    nc = build_nc()
    res = run_bass_kernel_spmd(nc, in_maps, list(range(NCORES)))
    results = res.results

    fcb_idio = np.asarray(inputs["fc_b"], np.float32)[16032:16532]
    idx = np.arange(NA)
    out = np.empty((B_FULL, NA, NA), np.float32)
    for core in range(NCORES):
        sigma = np.array(results[core]["sigma"], np.float32)
        idio = np.exp(np.asarray(results[core]["idio_raw"]) + fcb_idio[None, :])
        sigma[:, idx, idx] += idio.astype(np.float32)
        out[core * BL:(core + 1) * BL] = sigma
    return out



# revision 16
# speedup vs baseline: 1.0058x; 1.0058x over previous
"""Trainium2 Bass kernel for nn_FactorCovModel.

Model: 2-layer LSTM (H=512) over [B=256, T=64, D=500], last hidden ->
FC [512 -> 16532] -> Sigma = Lambda diag(exp(fv)) Lambda^T + diag(exp(idio)),
output [256, 500, 500].

Sharding: pure data parallel over batch, 32 samples/core on 8 cores.

Per-core device algorithm (matmul operands bf16, fp32 PSUM accumulation):
  - Weight gate axis host-permuted so PSUM col group hg holds hidden slice
    hg of ALL FOUR gates: PSUM [128 = (hg, batch), 512 = i|f|g|o x 128].
    Every ACT/DVE op is then full-128-partition and partition-aligned.
  - LSTM gates are computed column-tiled: stationary = hT chunk [128, 32],
    4 hidden-slice groups run concurrently at tile positions (0, 32j).
  - x-projection (xg0) matmuls accumulate into the same PSUM tile two
    steps ahead; recurrent matmuls then add onto it (start=False).
  - Layer-1 consumes h0T[t] directly (fused input projection, contraction
    [h0T; h1T] = 1024) plus a bias inject via a stacked-identity stationary.
  - FC runs col-packed (4 feature tiles of 512 per PSUM tile), then Lambda
    is re-laid-out via 500 PE transposes of [32, 32] blocks into
    LT [32 factors, 500 assets, 32 batch]; fvar gets exp via ACT.
  - Sigma_b = (LT_b * f_b)^T @ LT_b per sample, 4 m-tiles of 128.
  - idio raw rows go back to the host, which applies bias+exp and adds the
    diagonal (avoids diagonal APs on device).
"""

import os
import sys

sys.path.insert(0, "/opt/trn_rl_repo")

import numpy as np

import concourse.bass as bass
import concourse.mybir as mybir
from concourse import bacc
from concourse.tile import TileContext

FP = mybir.dt.float32
BF = mybir.dt.bfloat16
F16 = mybir.dt.float16
AF = mybir.ActivationFunctionType

B_FULL, T_FULL, D_IN, H = 256, 64, 500, 512
NCORES = 8
BL = B_FULL // NCORES            # 32 samples per core
NA, NF = 500, 32                 # assets, factors
OUT_DIM = NA * NF + NF + NA      # 16532
NTILE = 512                      # FC feature tile
N_FTILES = 33                    # ceil(16532/512) -> features padded to 16896
FH = N_FTILES * NTILE            # 16896
XCHUNK = 16                      # time steps per streamed xT chunk
N_PREF = 24                      # fcw feature tiles prefetched during LSTM

# gate-axis permutation: new col (hg, gate, hl) = 512*hg + 128*gate + hl maps to
# old row gate*512 + 128*hg + hl (torch gate order [i, f, g, o]).  With this
# layout, PSUM col group hg holds ALL FOUR gates of hidden slice hg along the
# free dim, so every ACT/DVE op is full-128-partition and partition-aligned.
PERM = np.array([gate * 512 + 128 * hg + hl
                 for hg in range(4) for gate in range(4) for hl in range(128)])


# ---------------------------------------------------------------- host prep

def host_prep_shared(inputs):
    w_ih0 = np.asarray(inputs["w_ih0"])[PERM]
    w_hh0 = np.asarray(inputs["w_hh0"])[PERM]
    b0 = (np.asarray(inputs["b_ih0"]) + np.asarray(inputs["b_hh0"]))[PERM]
    w_ih1 = np.asarray(inputs["w_ih1"])[PERM]
    w_hh1 = np.asarray(inputs["w_hh1"])[PERM]
    b1 = (np.asarray(inputs["b_ih1"]) + np.asarray(inputs["b_hh1"]))[PERM]
    fc_w = np.asarray(inputs["fc_w"])
    fc_b = np.asarray(inputs["fc_b"])

    w0T = np.zeros((512, 2048), np.float32)
    w0T[:500] = w_ih0.T
    w0T[500] = b0
    wh0T = np.ascontiguousarray(w_hh0.T, dtype=np.float32)
    w1T = np.ascontiguousarray(np.concatenate([w_ih1.T, w_hh1.T]), dtype=np.float32)
    b1T = np.zeros((128, 512), np.float32)
    for j in range(4):
        b1T[32 * j:32 * (j + 1), :] = b1[512 * j:512 * (j + 1)][None, :]
    fcwT = np.zeros((512, FH), np.float32)
    fcwT[:, :OUT_DIM] = fc_w.T
    fcbT = np.zeros((32, 512), np.float32)
    fcbT[:, :500] = fc_b[:16000].reshape(500, 32).T
    fcbF = np.ascontiguousarray(fc_b[16000:16032].reshape(32, 1), dtype=np.float32)
    ident = np.ascontiguousarray(np.tile(np.eye(32, dtype=np.float32), (4, 1)))
    import ml_dtypes
    tobf = lambda a: np.ascontiguousarray(a, dtype=ml_dtypes.bfloat16)
    return dict(w0T=tobf(w0T), wh0T=tobf(wh0T), w1T=tobf(w1T), b1T=b1T,
                fcwT=tobf(fcwT), fcbT=fcbT, fcbF=fcbF, identt=ident,
                identb=tobf(ident))


def host_prep_x(x_core):
    """x_core [BL, T, 500] -> xT [512, T*BL], (t, b) free order, ones bias row."""
    T = x_core.shape[1]
    import ml_dtypes
    xT = np.zeros((512, T * BL), np.float32)
    xT[:500] = np.asarray(x_core, np.float32).transpose(2, 1, 0).reshape(500, T * BL)
    xT[500] = 1.0
    return np.ascontiguousarray(xT, dtype=ml_dtypes.bfloat16)


# ---------------------------------------------------------------- bass build

def build_nc(T=T_FULL):
    nc = bacc.Bacc("TRN2")

    xT_d = nc.dram_tensor("xT", [512, T * BL], BF, kind="ExternalInput")
    w0T_d = nc.dram_tensor("w0T", [512, 2048], BF, kind="ExternalInput")
    wh0T_d = nc.dram_tensor("wh0T", [512, 2048], BF, kind="ExternalInput")
    w1T_d = nc.dram_tensor("w1T", [1024, 2048], BF, kind="ExternalInput")
    b1T_d = nc.dram_tensor("b1T", [128, 512], FP, kind="ExternalInput")
    fcwT_d = nc.dram_tensor("fcwT", [512, FH], BF, kind="ExternalInput")
    fcbT_d = nc.dram_tensor("fcbT", [32, 512], FP, kind="ExternalInput")
    fcbF_d = nc.dram_tensor("fcbF", [32, 1], FP, kind="ExternalInput")
    identt_d = nc.dram_tensor("identt", [128, 32], FP, kind="ExternalInput")
    identb_d = nc.dram_tensor("identb", [128, 32], BF, kind="ExternalInput")

    sigma_d = nc.dram_tensor("sigma", [BL, NA, NA], F16, kind="ExternalOutput")
    idio_d = nc.dram_tensor("idio_raw", [BL, NA], BF, kind="ExternalOutput")

    def mm(out, lhsT, rhs, tp, **kw):
        nc.tensor.matmul(out, lhsT, rhs,
                         tile_position=tp, skip_group_check=True, **kw)

    def tr(out, in_, identity, tp):
        nc.tensor.matmul(out, in_, identity, is_transpose=True,
                         tile_position=tp, skip_group_check=True)

    with TileContext(nc) as tc:
        with tc.tile_pool(name="persist", bufs=1) as persist:
            b1T_sb = persist.tile([128, 512], FP)
            nc.sync.dma_start(b1T_sb, b1T_d[:, :])
            identt_sb = persist.tile([128, 32], FP)
            nc.sync.dma_start(identt_sb, identt_d[:, :])
            identb_sb = persist.tile([128, 32], BF)
            nc.sync.dma_start(identb_sb, identb_d[:, :])
            fcbT_sb = persist.tile([32, 512], FP)
            nc.sync.dma_start(fcbT_sb, fcbT_d[:, :])
            fcbF_sb = persist.tile([32, 1], FP)
            nc.sync.dma_start(fcbF_sb, fcbF_d[:, :])
            hlast = persist.tile([128, 128], BF)  # final h1T, chunk-major cols
            # prefetch the first N_PREF fcw feature tiles during phase 1 (the
            # DMA engines are otherwise idle while the LSTM runs)
            fcw_pref = persist.tile([128, 4, N_PREF * 512], BF)
            nc.sync.dma_start(
                fcw_pref,
                fcwT_d[:, :N_PREF * 512].rearrange("(ko p) n -> p ko n", p=128),
            )

            # ---------------- phase 1: LSTM ----------------
            with (
                tc.tile_pool(name="wconst", bufs=1) as wconst,
                tc.tile_pool(name="xring", bufs=2) as xring,
                tc.tile_pool(name="state", bufs=2) as state,
                tc.tile_pool(name="work", bufs=2) as work,
                tc.tile_pool(name="pgates", bufs=8, space="PSUM") as pgates,
            ):
                w0T_sb = wconst.tile([128, 4, 2048], BF)
                nc.sync.dma_start(w0T_sb, w0T_d.rearrange("(ko p) g -> p ko g", p=128))
                wh0T_sb = wconst.tile([128, 4, 2048], BF)
                nc.sync.dma_start(wh0T_sb, wh0T_d.rearrange("(ko p) g -> p ko g", p=128))
                w1T_sb = wconst.tile([128, 8, 2048], BF)
                nc.sync.dma_start(w1T_sb, w1T_d.rearrange("(ko p) g -> p ko g", p=128))

                xch = min(XCHUNK, T)
                n_xchunks = (T + xch - 1) // xch
                x_tiles = {}

                def load_xchunk(ci):
                    if ci >= n_xchunks:
                        return
                    xt = xring.tile([128, 4, xch * BL], BF, tag="xchunk")
                    nc.sync.dma_start(
                        xt,
                        xT_d[:, ci * xch * BL:(ci + 1) * xch * BL]
                        .rearrange("(ko p) tb -> p ko tb", p=128),
                    )
                    x_tiles[ci] = xt

                load_xchunk(0)

                def gates_banks(nm):
                    # one PSUM bank per hidden-slice col group: concurrent
                    # col-tiled matmuls into the SAME bank corrupt on HW.
                    return [pgates.tile([128, 512], FP, tag="g", name=f"{nm}{j}")
                            for j in range(4)]

                def evac(pgs, dst, bias):
                    """Copy group j of pgs -> dst[32j:32j+32] (+bias), 2 ACT / 2 DVE."""
                    for j in range(4):
                        s = slice(32 * j, 32 * (j + 1))
                        if bias is None:
                            if j % 2 == 0:
                                nc.scalar.copy(dst[s, :], pgs[j][s, :])
                            else:
                                nc.vector.tensor_copy(dst[s, :], pgs[j][s, :])
                        else:
                            nc.vector.tensor_add(dst[s, :], pgs[j][s, :],
                                                 bias[s, :])
                    return dst

                def gate_nonlin(ga, cprev, cnew, tag):
                    """ga [128 = (hg, b), 512 = i|f|g|o x128] SBUF -> (H, cnew)."""
                    a = work.tile([128, 512], FP, tag=f"act_{tag}")
                    nc.scalar.activation(a[:, 0:256], ga[:, 0:256], AF.Sigmoid)
                    nc.scalar.activation(a[:, 256:384], ga[:, 256:384], AF.Tanh)
                    nc.scalar.activation(a[:, 384:512], ga[:, 384:512], AF.Sigmoid)
                    t1 = work.tile([128, 128], FP, tag=f"t1_{tag}")
                    nc.vector.tensor_mul(t1, a[:, 0:128], a[:, 256:384])
                    if cprev is None:
                        cn = t1  # c_prev == 0 at t == 0
                    else:
                        t2 = work.tile([128, 128], FP, tag=f"t2_{tag}")
                        nc.vector.tensor_mul(t2, a[:, 128:256], cprev)
                        cn = cnew
                        nc.vector.tensor_add(cn, t1, t2)
                    tcn = work.tile([128, 128], FP, tag=f"tc_{tag}")
                    nc.scalar.activation(tcn, cn, AF.Tanh)
                    hh = work.tile([128, 128], FP, tag=f"h_{tag}")
                    nc.vector.tensor_mul(hh, a[:, 384:512], tcn)
                    return hh, cn

                def transpose_h(hh, tag):
                    """hh [128=(hg,b),128] -> hT bf16 [128,128]; each 32-col
                    transpose gets its OWN psum bank (concurrent row-tiled
                    transposes into one bank corrupt on HW)."""
                    ht = state.tile([128, 128], BF, tag=f"ht_{tag}")
                    for k in range(4):
                        ptk = pgates.tile([128, 32], FP, tag="g", name=f"pt{k}")
                        tr(ptk, hh[32 * k:32 * (k + 1), :],
                           identt_sb[32 * k:32 * (k + 1), :], (32 * k, 0))
                        nc.vector.tensor_copy(ht[:, 32 * k:32 * (k + 1)], ptk)
                    return ht

                c0 = c1 = None
                h0T = h1T = None
                for t in range(T):
                    ci, tl = t // xch, t % xch
                    if tl == 0:
                        load_xchunk(ci + 1)
                    xt = x_tiles[ci]

                    pgs0 = gates_banks("g0_")
                    last0 = (t == 0)
                    for k in range(4):
                        lhsT = xt[:, k, tl * BL:(tl + 1) * BL]
                        for j in range(4):
                            mm(pgs0[j][32 * j:32 * (j + 1), :], lhsT,
                               w0T_sb[:, k, 512 * j:512 * (j + 1)],
                               tp=(0, 32 * j),
                               start=(k == 0), stop=(last0 and k == 3))
                    if t > 0:
                        for k in range(4):
                            lhsT = h0T[:, 32 * k:32 * (k + 1)]
                            for j in range(4):
                                mm(pgs0[j][32 * j:32 * (j + 1), :], lhsT,
                                   wh0T_sb[:, k, 512 * j:512 * (j + 1)],
                                   tp=(0, 32 * j),
                                   start=False, stop=(k == 3))
                    a0 = work.tile([128, 512], FP, tag="ga_l0")
                    evac(pgs0, a0, None)
                    c0n = None if c0 is None else state.tile([128, 128], FP, tag="c0")
                    h0, c0 = gate_nonlin(a0, c0, c0n, "l0")
                    h0T = transpose_h(h0, "l0")

                    pgs1 = gates_banks("g1_")
                    nk = 4 if t == 0 else 8
                    for k in range(nk):
                        srct = h0T if k < 4 else h1T
                        lhsT = srct[:, 32 * (k % 4):32 * (k % 4 + 1)]
                        for j in range(4):
                            mm(pgs1[j][32 * j:32 * (j + 1), :], lhsT,
                               w1T_sb[:, k, 512 * j:512 * (j + 1)],
                               tp=(0, 32 * j),
                               start=(k == 0), stop=(k == nk - 1))
                    a1 = work.tile([128, 512], FP, tag="ga_l1")
                    evac(pgs1, a1, b1T_sb)
                    c1n = None if c1 is None else state.tile([128, 128], FP, tag="c1")
                    h1, c1 = gate_nonlin(a1, c1, c1n, "l1")
                    h1T = transpose_h(h1, "l1")

                nc.vector.tensor_copy(hlast, h1T)

            # ---------------- phase 2: FC + Lambda layout + Sigma ----------------
            with (
                tc.tile_pool(name="fcw", bufs=3) as fcwp,
                tc.tile_pool(name="rawp", bufs=3) as rawp,
                tc.tile_pool(name="lt", bufs=1) as ltp,
                tc.tile_pool(name="sigw", bufs=4) as sigw,
                tc.tile_pool(name="pfc", bufs=4, space="PSUM") as pfcp,
                tc.tile_pool(name="plt", bufs=2, space="PSUM") as pltp,
                tc.tile_pool(name="psig", bufs=2, space="PSUM") as psigp,
            ):
                LT = ltp.tile([32, 500, 32], BF)       # [factor, asset, b]
                F_sb = ltp.tile([32, 32], FP)          # exp(fvar raw + bias) [factor, b]

                n_quads = (N_FTILES + 3) // 4          # 9 (last quad has 1 tile)
                for q in range(n_quads):
                    rr = range(4) if q < 8 else range(1)
                    raw_t = rawp.tile([128, 512], BF, tag="raw")
                    for r in rr:
                        jj = 4 * q + r
                        if jj < N_PREF:
                            fcw_src = fcw_pref[:, :, jj * 512:(jj + 1) * 512]
                        else:
                            fcw_t = fcwp.tile([128, 4, 512], BF, tag="fcw")
                            nc.sync.dma_start(
                                fcw_t,
                                fcwT_d[:, jj * 512:(jj + 1) * 512]
                                .rearrange("(ko p) n -> p ko n", p=128),
                            )
                            fcw_src = fcw_t
                        # one PSUM bank per n-tile (col groups must not share)
                        pfc = pfcp.tile([128, 512], FP, tag="pfc")
                        for k in range(4):
                            mm(pfc[32 * r:32 * (r + 1), :],
                               hlast[:, 32 * k:32 * (k + 1)],
                               fcw_src[:, k, :],
                               tp=(0, 32 * r),
                               start=(k == 0), stop=(k == 3))
                        s = slice(32 * r, 32 * (r + 1))
                        if r % 2 == 0:
                            nc.scalar.copy(raw_t[s, :], pfc[s, :])
                        else:
                            nc.vector.tensor_copy(raw_t[s, :], pfc[s, :])

                    # Lambda blocks inside this quad -> transpose into LT
                    for r in rr:
                        jj = 4 * q + r
                        base_feat = jj * 512
                        nblk = 16 if jj < 31 else (4 if jj == 31 else 0)
                        for g in range(0, nblk, 4):
                            pt = pltp.tile([32, 128], BF, tag="plt")
                            for a in range(4):
                                blk = g + a
                                tr(pt[:, 32 * a:32 * (a + 1)],
                                   raw_t[32 * r:32 * (r + 1),
                                         32 * blk:32 * (blk + 1)],
                                   identb_sb[32 * r:32 * (r + 1), :], (32 * r, 0))
                            a0 = (base_feat + 32 * g) // 32  # first asset index
                            nc.vector.tensor_tensor(
                                LT[:, a0:a0 + 4, :],
                                pt.rearrange("f (a b) -> f a b", a=4),
                                fcbT_sb[:, a0:a0 + 4, None].to_broadcast([32, 4, 32]),
                                mybir.AluOpType.add,
                            )
                        if jj == 31:
                            # fvar: features 16000:16032 = cols 128:160 (r == 3)
                            ptf_full = pltp.tile([32, 128], BF, tag="plt")
                            ptf = ptf_full[:, 0:32]
                            tr(ptf, raw_t[96:128, 128:160],
                               identb_sb[96:128, :], (96, 0))
                            nc.scalar.activation(F_sb, ptf, AF.Exp,
                                                 bias=fcbF_sb[:, 0:1])
                            # idio part 1: features 16032:16384 = cols 160:512
                            nc.sync.dma_start(idio_d[:, 0:352],
                                              raw_t[96:128, 160:512])
                        if jj == 32:
                            # idio part 2: features 16384:16532 = cols 0:148
                            nc.sync.dma_start(idio_d[:, 352:500],
                                              raw_t[0:32, 0:148])

                # Sigma per sample pair: 4 m-tiles of 125 rows, staged fp16,
                # one fused DMA per pair (cuts trigger count 8x and bytes 2x)
                for b0 in range(0, BL, 2):
                    st2 = sigw.tile([128, 2, 4, 500], F16, tag="sigstage")
                    for bb in range(2):
                        b = b0 + bb
                        gt = sigw.tile([32, 512], BF, tag="gt")
                        nc.vector.tensor_scalar_mul(gt[:, 0:500], LT[:, :, b],
                                                    F_sb[:, b:b + 1])
                        for mt in range(4):
                            ps = psigp.tile([128, 512], FP, tag="psig")
                            mm(ps[:125, 0:500], gt[:, 125 * mt:125 * mt + 125],
                               LT[:, :, b], tp=(0, 0), start=True, stop=True)
                            if mt % 2 == 0:
                                nc.scalar.copy(st2[:125, bb, mt, :],
                                               ps[:125, 0:500])
                            else:
                                nc.vector.tensor_copy(st2[:125, bb, mt, :],
                                                      ps[:125, 0:500])
                    nc.sync.dma_start(
                        sigma_d[b0:b0 + 2].rearrange("b (m p) n -> p b m n",
                                                     p=125),
                        st2[:125])

    nc.compile()
    return nc


# ---------------------------------------------------------------- entry point

def kernel(**inputs):
    from concourse.bass_utils import run_bass_kernel_spmd

    prep = host_prep_shared(inputs)
    x = np.asarray(inputs["x"], np.float32)
    in_maps = []
    for core in range(NCORES):
        m = dict(prep)
        m["xT"] = host_prep_x(x[core * BL:(core + 1) * BL])
        in_maps.append(m)

    nc = build_nc()
    res = run_bass_kernel_spmd(nc, in_maps, list(range(NCORES)))
    results = res.results

    fcb_idio = np.asarray(inputs["fc_b"], np.float32)[16032:16532]
    idx = np.arange(NA)
    out = np.empty((B_FULL, NA, NA), np.float32)
    for core in range(NCORES):
        sigma = np.asarray(results[core]["sigma"]).astype(np.float32)
        idio = np.exp(np.asarray(results[core]["idio_raw"], np.float32)
                      + fcb_idio[None, :])
        sigma[:, idx, idx] += idio
        out[core * BL:(core + 1) * BL] = sigma
    return out



# revision 26
# speedup vs baseline: 1.0622x; 1.0561x over previous
"""Trainium2 Bass kernel for nn_FactorCovModel.

Model: 2-layer LSTM (H=512) over [B=256, T=64, D=500], last hidden ->
FC [512 -> 16532] -> Sigma = Lambda diag(exp(fv)) Lambda^T + diag(exp(idio)),
output [256, 500, 500].

Sharding: pure data parallel over batch, 32 samples/core on 8 cores.

Per-core device algorithm (matmul operands bf16, fp32 PSUM accumulation):
  - Weight gate axis host-permuted so PSUM col group hg holds hidden slice
    hg of ALL FOUR gates: PSUM [128 = (hg, batch), 512 = i|f|g|o x 128].
    Every ACT/DVE op is then full-128-partition and partition-aligned.
  - LSTM gates are computed column-tiled: stationary = hT chunk [128, 32],
    4 hidden-slice groups run concurrently at tile positions (0, 32j).
  - x-projection (xg0) matmuls accumulate into the same PSUM tile two
    steps ahead; recurrent matmuls then add onto it (start=False).
  - Layer-1 consumes h0T[t] directly (fused input projection, contraction
    [h0T; h1T] = 1024) plus a bias inject via a stacked-identity stationary.
  - FC runs col-packed (4 feature tiles of 512 per PSUM tile), then Lambda
    is re-laid-out via 500 PE transposes of [32, 32] blocks into
    LT [32 factors, 500 assets, 32 batch]; fvar gets exp via ACT.
  - Sigma_b = (LT_b * f_b)^T @ LT_b per sample, 4 m-tiles of 128.
  - idio raw rows go back to the host, which applies bias+exp and adds the
    diagonal (avoids diagonal APs on device).
"""

import os
import sys

sys.path.insert(0, "/opt/trn_rl_repo")

import numpy as np

import concourse.bass as bass
import concourse.mybir as mybir
from concourse import bacc
from concourse.tile import TileContext

FP = mybir.dt.float32
BF = mybir.dt.bfloat16
F16 = mybir.dt.float16
AF = mybir.ActivationFunctionType

B_FULL, T_FULL, D_IN, H = 256, 64, 500, 512
NCORES = 8
BL = B_FULL // NCORES            # 32 samples per core
NA, NF = 500, 32                 # assets, factors
OUT_DIM = NA * NF + NF + NA      # 16532
NTILE = 512                      # FC feature tile
N_FTILES = 33                    # ceil(16532/512) -> features padded to 16896
FH = N_FTILES * NTILE            # 16896
XCHUNK = 16                      # time steps per streamed xT chunk
N_PREF = 8                       # fcw feature tiles prefetched during LSTM

# gate-axis permutation: new col (hg, gate, hl) = 512*hg + 128*gate + hl maps to
# old row gate*512 + 128*hg + hl (torch gate order [i, f, g, o]).  With this
# layout, PSUM col group hg holds ALL FOUR gates of hidden slice hg along the
# free dim, so every ACT/DVE op is full-128-partition and partition-aligned.
PERM = np.array([gate * 512 + 128 * hg + hl
                 for hg in range(4) for gate in range(4) for hl in range(128)])


# ---------------------------------------------------------------- host prep

def host_prep_shared(inputs):
    w_ih0 = np.asarray(inputs["w_ih0"])[PERM]
    w_hh0 = np.asarray(inputs["w_hh0"])[PERM]
    b0 = (np.asarray(inputs["b_ih0"]) + np.asarray(inputs["b_hh0"]))[PERM]
    w_ih1 = np.asarray(inputs["w_ih1"])[PERM]
    w_hh1 = np.asarray(inputs["w_hh1"])[PERM]
    b1 = (np.asarray(inputs["b_ih1"]) + np.asarray(inputs["b_hh1"]))[PERM]
    fc_w = np.asarray(inputs["fc_w"])
    fc_b = np.asarray(inputs["fc_b"])

    w0T = np.zeros((512, 2048), np.float32)
    w0T[:500] = w_ih0.T
    w0T[500] = b0
    wh0T = np.ascontiguousarray(w_hh0.T, dtype=np.float32)
    w1xT = np.ascontiguousarray(w_ih1.T, dtype=np.float32)
    wh1T = np.ascontiguousarray(w_hh1.T, dtype=np.float32)
    b1row = np.ascontiguousarray(b1[None, :], dtype=np.float32)
    ones1 = np.ones((1, 128), np.float32)
    fcwT = np.zeros((512, FH), np.float32)
    fcwT[:, :OUT_DIM] = fc_w.T
    fcbT = np.zeros((32, 512), np.float32)
    fcbT[:, :500] = fc_b[:16000].reshape(500, 32).T
    fcbF = np.ascontiguousarray(fc_b[16000:16032].reshape(32, 1), dtype=np.float32)
    ident = np.ascontiguousarray(np.tile(np.eye(32, dtype=np.float32), (4, 1)))
    import ml_dtypes
    tobf = lambda a: np.ascontiguousarray(a, dtype=ml_dtypes.bfloat16)
    return dict(w0T=tobf(w0T), wh0T=tobf(wh0T), w1xT=tobf(w1xT),
                wh1T=tobf(wh1T), b1row=tobf(b1row), ones1=tobf(ones1),
                fcwT=tobf(fcwT), fcbT=fcbT, fcbF=fcbF, identt=ident,
                identb=tobf(ident))


def host_prep_x(x_core):
    """x_core [BL, T, 500] -> xT [512, T*BL], (t, b) free order, ones bias row."""
    T = x_core.shape[1]
    import ml_dtypes
    xT = np.zeros((512, T * BL), np.float32)
    xT[:500] = np.asarray(x_core, np.float32).transpose(2, 1, 0).reshape(500, T * BL)
    xT[500] = 1.0
    return np.ascontiguousarray(xT, dtype=ml_dtypes.bfloat16)


# ---------------------------------------------------------------- bass build

def build_nc(T=T_FULL):
    nc = bacc.Bacc("TRN2")

    xT_d = nc.dram_tensor("xT", [512, T * BL], BF, kind="ExternalInput")
    w0T_d = nc.dram_tensor("w0T", [512, 2048], BF, kind="ExternalInput")
    wh0T_d = nc.dram_tensor("wh0T", [512, 2048], BF, kind="ExternalInput")
    w1xT_d = nc.dram_tensor("w1xT", [512, 2048], BF, kind="ExternalInput")
    wh1T_d = nc.dram_tensor("wh1T", [512, 2048], BF, kind="ExternalInput")
    b1row_d = nc.dram_tensor("b1row", [1, 2048], BF, kind="ExternalInput")
    ones1_d = nc.dram_tensor("ones1", [1, 128], BF, kind="ExternalInput")
    fcwT_d = nc.dram_tensor("fcwT", [512, FH], BF, kind="ExternalInput")
    fcbT_d = nc.dram_tensor("fcbT", [32, 512], FP, kind="ExternalInput")
    fcbF_d = nc.dram_tensor("fcbF", [32, 1], FP, kind="ExternalInput")
    identt_d = nc.dram_tensor("identt", [128, 32], FP, kind="ExternalInput")
    identb_d = nc.dram_tensor("identb", [128, 32], BF, kind="ExternalInput")

    sigma_d = nc.dram_tensor("sigma", [BL, NA, NA], F16, kind="ExternalOutput")
    idio_d = nc.dram_tensor("idio_raw", [BL, NA], BF, kind="ExternalOutput")

    def mm(out, lhsT, rhs, tp, **kw):
        nc.tensor.matmul(out, lhsT, rhs,
                         tile_position=tp, skip_group_check=True, **kw)

    def tr(out, in_, identity, tp):
        nc.tensor.matmul(out, in_, identity, is_transpose=True,
                         tile_position=tp, skip_group_check=True)

    with TileContext(nc) as tc:
        with tc.tile_pool(name="persist", bufs=1) as persist:
            ones1_sb = persist.tile([1, 128], BF)
            nc.sync.dma_start(ones1_sb, ones1_d[:, :])
            b1row_sb = persist.tile([1, 2048], BF)
            nc.sync.dma_start(b1row_sb, b1row_d[:, :])
            identt_sb = persist.tile([128, 32], FP)
            nc.sync.dma_start(identt_sb, identt_d[:, :])
            identb_sb = persist.tile([128, 32], BF)
            nc.sync.dma_start(identb_sb, identb_d[:, :])
            fcbT_sb = persist.tile([32, 512], FP)
            nc.sync.dma_start(fcbT_sb, fcbT_d[:, :])
            fcbF_sb = persist.tile([32, 1], FP)
            nc.sync.dma_start(fcbF_sb, fcbF_d[:, :])
            hlast = persist.tile([128, 128], BF)  # final h1T, chunk-major cols
            # prefetched fcw feature tiles (DMA issued after the LSTM weight
            # loads so it doesn't delay the LSTM start)
            fcw_pref = persist.tile([128, 4, N_PREF * 512], BF)

            # ---------------- phase 1: LSTM ----------------
            # Both layers' input projections run once per 4-step BLOCK with a
            # full [128,128] stationary (weights stream 1x per block instead
            # of per step).  Block outputs land in PSUM with rows (m, b)
            # (m = step-in-block); per-(m, j) shift-DMAs re-align them to the
            # per-step (hidden-group, b) gate layout in SBUF (xga).  The
            # per-step evac then adds xga onto the recurrent PSUM gates.
            # Layer 1 consumes blocked h0 projections SKEW steps behind l0.
            SKEW = 5
            with (
                tc.tile_pool(name="wconst", bufs=1) as wconst,
                tc.tile_pool(name="xring", bufs=2) as xring,
                tc.tile_pool(name="state", bufs=2) as state,
                tc.tile_pool(name="xgap", bufs=2) as xgap,
                tc.tile_pool(name="work", bufs=2) as work,
                tc.tile_pool(name="pgates", bufs=8, space="PSUM") as pgates,
            ):
                w0T_sb = wconst.tile([128, 4, 2048], BF)
                nc.sync.dma_start(w0T_sb, w0T_d.rearrange("(ko p) g -> p ko g", p=128))
                wh0T_sb = wconst.tile([128, 4, 2048], BF)
                nc.sync.dma_start(wh0T_sb, wh0T_d.rearrange("(ko p) g -> p ko g", p=128))
                w1x_sb = wconst.tile([128, 4, 2048], BF)
                nc.sync.dma_start(w1x_sb, w1xT_d.rearrange("(ko p) g -> p ko g", p=128))
                wh1_sb = wconst.tile([128, 4, 2048], BF)
                nc.sync.dma_start(wh1_sb, wh1T_d.rearrange("(ko p) g -> p ko g", p=128))

                xch = min(XCHUNK, T)
                n_xchunks = (T + xch - 1) // xch
                n_blocks = T // 4
                x_tiles = {}

                def load_xchunk(ci):
                    if ci >= n_xchunks:
                        return
                    xt = xring.tile([128, 4, xch * BL], BF, tag="xchunk")
                    nc.sync.dma_start(
                        xt,
                        xT_d[:, ci * xch * BL:(ci + 1) * xch * BL]
                        .rearrange("(ko p) tb -> p ko tb", p=128),
                    )
                    x_tiles[ci] = xt

                load_xchunk(0)
                # fcw prefetch rides the idle DMA bandwidth behind the weights
                nc.scalar.dma_start(
                    fcw_pref,
                    fcwT_d[:, :N_PREF * 512].rearrange("(ko p) n -> p ko n",
                                                       p=128),
                )

                def emit_xp_block(which, B, h0blk_src=None):
                    """Blocked input projection for steps [4B, 4B+4).
                    which=0: x @ w0T (bias via xT ones row);
                    which=1: h0 @ w1xT + b1 (bias via 1-row matmul).
                    Returns xga [128=(j,b), 4=m, 512] in SBUF."""
                    xpps = [pgates.tile([128, 512], FP, tag="g",
                                        name=f"xpp{which}_{j}")
                            for j in range(4)]
                    if which == 0:
                        xt = x_tiles[B // 4]
                        rel = B % 4
                        for k in range(4):
                            lhsT = xt[:, k, rel * 128:(rel + 1) * 128]
                            for j in range(4):
                                mm(xpps[j][:, :], lhsT,
                                   w0T_sb[:, k, 512 * j:512 * (j + 1)],
                                   tp=(0, 0), start=(k == 0), stop=(k == 3))
                    else:
                        for j in range(4):
                            mm(xpps[j][:, :], ones1_sb[0:1, :],
                               b1row_sb[0:1, 512 * j:512 * (j + 1)],
                               tp=(0, 0), start=True, stop=False)
                        for k in range(4):
                            lhsT = h0blk_src[:, k, :]
                            for j in range(4):
                                mm(xpps[j][:, :], lhsT,
                                   w1x_sb[:, k, 512 * j:512 * (j + 1)],
                                   tp=(0, 0), start=False, stop=(k == 3))
                    # PSUM -> SBUF block evac (DMA cannot read PSUM), then
                    # partition-shifting DMAs build the per-step layout
                    xgb = xgap.tile([128, 4, 512], BF, tag=f"xgb{which}")
                    for j in range(4):
                        if j % 2 == 0:
                            nc.scalar.copy(xgb[:, j, :], xpps[j][:, :])
                        else:
                            nc.vector.tensor_copy(xgb[:, j, :], xpps[j][:, :])
                    xga = xgap.tile([128, 4, 512], BF, tag=f"xga{which}")
                    for m_ in range(4):
                        for j in range(4):
                            nc.sync.dma_start(
                                xga[32 * j:32 * (j + 1), m_, :],
                                xgb[32 * m_:32 * (m_ + 1), j, :])
                    return xga

                def gates_banks(nm):
                    # one PSUM bank per hidden-slice col group: concurrent
                    # col-tiled matmuls into the SAME bank corrupt on HW.
                    return [pgates.tile([128, 512], FP, tag="g", name=f"{nm}{j}")
                            for j in range(4)]

                def evac(pgs, dst, xga, m_):
                    """dst[32j:+32] = pgs[j][32j:+32] + xga[32j:+32, m_, :].
                    Split DVE TT / ACT copy + GpSimd add to balance engines."""
                    for j in range(4):
                        s = slice(32 * j, 32 * (j + 1))
                        if pgs is None:
                            if j % 2 == 0:
                                nc.scalar.copy(dst[s, :], xga[s, m_, :])
                            else:
                                nc.vector.tensor_copy(dst[s, :], xga[s, m_, :])
                        elif j < 2:
                            nc.vector.tensor_add(dst[s, :], pgs[j][s, :],
                                                 xga[s, m_, :])
                        else:
                            nc.scalar.copy(dst[s, :], pgs[j][s, :])
                            nc.gpsimd.tensor_add(dst[s, :], dst[s, :],
                                                 xga[s, m_, :])
                    return dst

                def gate_nonlin(ga, cprev, cnew, tag):
                    """ga [128 = (hg, b), 512 = i|f|g|o x128] SBUF -> (H, cnew)."""
                    a = work.tile([128, 512], FP, tag=f"act_{tag}")
                    nc.scalar.activation(a[:, 0:256], ga[:, 0:256], AF.Sigmoid)
                    nc.scalar.activation(a[:, 256:384], ga[:, 256:384], AF.Tanh)
                    nc.scalar.activation(a[:, 384:512], ga[:, 384:512], AF.Sigmoid)
                    t1 = work.tile([128, 128], FP, tag=f"t1_{tag}")
                    nc.vector.tensor_mul(t1, a[:, 0:128], a[:, 256:384])
                    if cprev is None:
                        cn = t1  # c_prev == 0 at t == 0
                    else:
                        t2 = work.tile([128, 128], FP, tag=f"t2_{tag}")
                        nc.vector.tensor_mul(t2, a[:, 128:256], cprev)
                        cn = cnew
                        nc.vector.tensor_add(cn, t1, t2)
                    tcn = work.tile([128, 128], FP, tag=f"tc_{tag}")
                    nc.scalar.activation(tcn, cn, AF.Tanh)
                    hh = work.tile([128, 128], FP, tag=f"h_{tag}")
                    nc.vector.tensor_mul(hh, a[:, 384:512], tcn)
                    return hh, cn

                def transpose_cols(hh, dst_cols, tag):
                    """hh [128=(hg,b),128] -> bf16 transposed into dst_cols[k]
                    [128, 32] slices; each 32-col transpose gets its OWN psum
                    bank (concurrent row-tiled transposes corrupt on HW)."""
                    for k in range(4):
                        ptk = pgates.tile([128, 32], FP, tag="g",
                                          name=f"pt{tag}{k}")
                        tr(ptk, hh[32 * k:32 * (k + 1), :],
                           identt_sb[32 * k:32 * (k + 1), :], (32 * k, 0))
                        nc.vector.tensor_copy(dst_cols[k], ptk)

                c0 = c1 = None
                h1T = None
                h0blk_cur = h0blk_prev = None
                xga0 = {}
                xga1 = {}
                xga0[0] = emit_xp_block(0, 0)
                for s in range(T + SKEW):
                    # ---- all matmuls first: the PE queue is in-order, so
                    # transposes (which wait on ACT/DVE) must not sit ahead
                    # of independent rec matmuls
                    pgs0 = pgs1 = None
                    if s < T:
                        m, B = s % 4, s // 4
                        if m == 0:
                            h0blk_prev = h0blk_cur
                            h0blk_cur = state.tile([128, 4, 128], BF,
                                                   tag="h0blk")
                            if B % 4 == 2:
                                # next x chunk, 5 supersteps before the first
                                # xp block emission that reads it
                                load_xchunk(B // 4 + 1)
                        if s > 0:
                            pgs0 = gates_banks("g0_")
                            src = h0blk_cur if m > 0 else h0blk_prev
                            mp = (s - 1) % 4
                            for k in range(4):
                                lhsT = src[:, k, 32 * mp:32 * (mp + 1)]
                                for j in range(4):
                                    mm(pgs0[j][32 * j:32 * (j + 1), :], lhsT,
                                       wh0T_sb[:, k, 512 * j:512 * (j + 1)],
                                       tp=(0, 32 * j),
                                       start=(k == 0), stop=(k == 3))
                    if s >= SKEW:
                        t1_ = s - SKEW
                        m1, B1 = t1_ % 4, t1_ // 4
                        if t1_ > 0:
                            pgs1 = gates_banks("g1_")
                            for k in range(4):
                                lhsT = h1T[:, 32 * k:32 * (k + 1)]
                                for j in range(4):
                                    mm(pgs1[j][32 * j:32 * (j + 1), :], lhsT,
                                       wh1_sb[:, k, 512 * j:512 * (j + 1)],
                                       tp=(0, 32 * j),
                                       start=(k == 0), stop=(k == 3))

                    # ---- layer-0 post-processing
                    if s < T:
                        a0 = work.tile([128, 512], FP, tag="ga_l0")
                        evac(pgs0, a0, xga0[B], m)
                        c0n = None if c0 is None else state.tile(
                            [128, 128], FP, tag="c0")
                        h0, c0 = gate_nonlin(a0, c0, c0n, "l0")
                        transpose_cols(
                            h0, [h0blk_cur[:, k, 32 * m:32 * (m + 1)]
                                 for k in range(4)], "l0")

                    # ---- layer-1 post-processing
                    if s >= SKEW:
                        a1 = work.tile([128, 512], FP, tag="ga_l1")
                        evac(pgs1, a1, xga1[B1], m1)
                        c1n = None if c1 is None else state.tile(
                            [128, 128], FP, tag="c1")
                        h1, c1 = gate_nonlin(a1, c1, c1n, "l1")
                        h1T_n = state.tile([128, 128], BF, tag="ht_l1")
                        transpose_cols(
                            h1, [h1T_n[:, 32 * k:32 * (k + 1)]
                                 for k in range(4)], "l1")
                        h1T = h1T_n

                    # ---- blocked input projections for upcoming steps
                    if s < T:
                        if m == 1 and B + 1 < n_blocks:
                            xga0[B + 1] = emit_xp_block(0, B + 1)
                        if m == 3:
                            xga1[B] = emit_xp_block(1, B, h0blk_cur)

                nc.vector.tensor_copy(hlast, h1T)

            # ---------------- phase 2: FC + Lambda layout + Sigma ----------------
            with (
                tc.tile_pool(name="fcw", bufs=3) as fcwp,
                tc.tile_pool(name="rawp", bufs=3) as rawp,
                tc.tile_pool(name="lt", bufs=1) as ltp,
                tc.tile_pool(name="sigw", bufs=4) as sigw,
                tc.tile_pool(name="pfc", bufs=4, space="PSUM") as pfcp,
                tc.tile_pool(name="plt", bufs=2, space="PSUM") as pltp,
                tc.tile_pool(name="psig", bufs=2, space="PSUM") as psigp,
            ):
                LT = ltp.tile([32, 500, 32], BF)       # [factor, asset, b]
                F_sb = ltp.tile([32, 32], FP)          # exp(fvar raw + bias) [factor, b]

                n_quads = (N_FTILES + 3) // 4          # 9 (last quad has 1 tile)
                for q in range(n_quads):
                    rr = range(4) if q < 8 else range(1)
                    raw_t = rawp.tile([128, 512], BF, tag="raw")
                    for r in rr:
                        jj = 4 * q + r
                        if jj < N_PREF:
                            fcw_src = fcw_pref[:, :, jj * 512:(jj + 1) * 512]
                        else:
                            fcw_t = fcwp.tile([128, 4, 512], BF, tag="fcw")
                            nc.sync.dma_start(
                                fcw_t,
                                fcwT_d[:, jj * 512:(jj + 1) * 512]
                                .rearrange("(ko p) n -> p ko n", p=128),
                            )
                            fcw_src = fcw_t
                        # one PSUM bank per n-tile (col groups must not share)
                        pfc = pfcp.tile([128, 512], FP, tag="pfc")
                        for k in range(4):
                            mm(pfc[32 * r:32 * (r + 1), :],
                               hlast[:, 32 * k:32 * (k + 1)],
                               fcw_src[:, k, :],
                               tp=(0, 32 * r),
                               start=(k == 0), stop=(k == 3))
                        s = slice(32 * r, 32 * (r + 1))
                        if r % 2 == 0:
                            nc.scalar.copy(raw_t[s, :], pfc[s, :])
                        else:
                            nc.vector.tensor_copy(raw_t[s, :], pfc[s, :])

                    # Lambda blocks inside this quad -> transpose into LT
                    for r in rr:
                        jj = 4 * q + r
                        base_feat = jj * 512
                        nblk = 16 if jj < 31 else (4 if jj == 31 else 0)
                        for g in range(0, nblk, 4):
                            pt = pltp.tile([32, 128], BF, tag="plt")
                            for a in range(4):
                                blk = g + a
                                tr(pt[:, 32 * a:32 * (a + 1)],
                                   raw_t[32 * r:32 * (r + 1),
                                         32 * blk:32 * (blk + 1)],
                                   identb_sb[32 * r:32 * (r + 1), :], (32 * r, 0))
                            a0 = (base_feat + 32 * g) // 32  # first asset index
                            nc.vector.tensor_tensor(
                                LT[:, a0:a0 + 4, :],
                                pt.rearrange("f (a b) -> f a b", a=4),
                                fcbT_sb[:, a0:a0 + 4, None].to_broadcast([32, 4, 32]),
                                mybir.AluOpType.add,
                            )
                        if jj == 31:
                            # fvar: features 16000:16032 = cols 128:160 (r == 3)
                            ptf_full = pltp.tile([32, 128], BF, tag="plt")
                            ptf = ptf_full[:, 0:32]
                            tr(ptf, raw_t[96:128, 128:160],
                               identb_sb[96:128, :], (96, 0))
                            nc.scalar.activation(F_sb, ptf, AF.Exp,
                                                 bias=fcbF_sb[:, 0:1])
                            # idio part 1: features 16032:16384 = cols 160:512
                            nc.sync.dma_start(idio_d[:, 0:352],
                                              raw_t[96:128, 160:512])
                        if jj == 32:
                            # idio part 2: features 16384:16532 = cols 0:148
                            nc.sync.dma_start(idio_d[:, 352:500],
                                              raw_t[0:32, 0:148])

                # Sigma per sample pair: 4 m-tiles of 125 rows, staged fp16,
                # one fused DMA per pair (cuts trigger count 8x and bytes 2x)
                for b0 in range(0, BL, 2):
                    st2 = sigw.tile([128, 2, 4, 500], F16, tag="sigstage")
                    for bb in range(2):
                        b = b0 + bb
                        gt = sigw.tile([32, 512], BF, tag="gt")
                        nc.vector.tensor_scalar_mul(gt[:, 0:500], LT[:, :, b],
                                                    F_sb[:, b:b + 1])
                        for mt in range(4):
                            ps = psigp.tile([128, 512], FP, tag="psig")
                            mm(ps[:125, 0:500], gt[:, 125 * mt:125 * mt + 125],
                               LT[:, :, b], tp=(0, 0), start=True, stop=True)
                            if mt % 2 == 0:
                                nc.scalar.copy(st2[:125, bb, mt, :],
                                               ps[:125, 0:500])
                            else:
                                nc.vector.tensor_copy(st2[:125, bb, mt, :],
                                                      ps[:125, 0:500])
                    nc.sync.dma_start(
                        sigma_d[b0:b0 + 2].rearrange("b (m p) n -> p b m n",
                                                     p=125),
                        st2[:125])

    nc.compile()
    return nc


# ---------------------------------------------------------------- entry point

def kernel(**inputs):
    from concourse.bass_utils import run_bass_kernel_spmd

    prep = host_prep_shared(inputs)
    x = np.asarray(inputs["x"], np.float32)
    in_maps = []
    for core in range(NCORES):
        m = dict(prep)
        m["xT"] = host_prep_x(x[core * BL:(core + 1) * BL])
        in_maps.append(m)

    nc = build_nc()
    res = run_bass_kernel_spmd(nc, in_maps, list(range(NCORES)))
    results = res.results

    fcb_idio = np.asarray(inputs["fc_b"], np.float32)[16032:16532]
    idx = np.arange(NA)
    out = np.empty((B_FULL, NA, NA), np.float32)
    for core in range(NCORES):
        sigma = np.asarray(results[core]["sigma"]).astype(np.float32)
        idio = np.exp(np.asarray(results[core]["idio_raw"], np.float32)
                      + fcb_idio[None, :])
        sigma[:, idx, idx] += idio
        out[core * BL:(core + 1) * BL] = sigma
    return out



# revision 38
# speedup vs baseline: 1.2011x; 1.1307x over previous
"""Trainium2 Bass kernel for nn_FactorCovModel.

Model: 2-layer LSTM (H=512) over [B=256, T=64, D=500], last hidden ->
FC [512 -> 16532] -> Sigma = Lambda diag(exp(fv)) Lambda^T + diag(exp(idio)),
output [256, 500, 500].

Sharding: pure data parallel over batch, 32 samples/core on 8 cores.

Per-core device algorithm (matmul operands bf16, fp32 PSUM accumulation):
  - Weight gate axis host-permuted so PSUM col group hg holds hidden slice
    hg of ALL FOUR gates: PSUM [128 = (hg, batch), 512 = i|f|g|o x 128].
    Every ACT/DVE op is then full-128-partition and partition-aligned.
  - LSTM gates are computed column-tiled: stationary = hT chunk [128, 32],
    4 hidden-slice groups run concurrently at tile positions (0, 32j).
  - x-projection (xg0) matmuls accumulate into the same PSUM tile two
    steps ahead; recurrent matmuls then add onto it (start=False).
  - Layer-1 consumes h0T[t] directly (fused input projection, contraction
    [h0T; h1T] = 1024) plus a bias inject via a stacked-identity stationary.
  - FC runs col-packed (4 feature tiles of 512 per PSUM tile), then Lambda
    is re-laid-out via 500 PE transposes of [32, 32] blocks into
    LT [32 factors, 500 assets, 32 batch]; fvar gets exp via ACT.
  - Sigma_b = (LT_b * f_b)^T @ LT_b per sample, 4 m-tiles of 128.
  - idio raw rows go back to the host, which applies bias+exp and adds the
    diagonal (avoids diagonal APs on device).
"""

import os
import sys

sys.path.insert(0, "/opt/trn_rl_repo")

import numpy as np

import concourse.bass as bass
import concourse.mybir as mybir
from concourse import bacc
from concourse.tile import TileContext

FP = mybir.dt.float32
BF = mybir.dt.bfloat16
F16 = mybir.dt.float16
AF = mybir.ActivationFunctionType

B_FULL, T_FULL, D_IN, H = 256, 64, 500, 512
NCORES = 8
BL = B_FULL // NCORES            # 32 samples per core
NA, NF = 500, 32                 # assets, factors
OUT_DIM = NA * NF + NF + NA      # 16532
NTILE = 512                      # FC feature tile
N_FTILES = 33                    # ceil(16532/512) -> features padded to 16896
FH = N_FTILES * NTILE            # 16896
XCHUNK = 16                      # time steps per streamed xT chunk
N_PREF = 8                       # fcw feature tiles prefetched during LSTM

# gate-axis permutation: new col (hg, gate, hl) = 512*hg + 128*gate + hl maps to
# old row gate*512 + 128*hg + hl (torch gate order [i, f, g, o]).  With this
# layout, PSUM col group hg holds ALL FOUR gates of hidden slice hg along the
# free dim, so every ACT/DVE op is full-128-partition and partition-aligned.
PERM = np.array([gate * 512 + 128 * hg + hl
                 for hg in range(4) for gate in range(4) for hl in range(128)])


# ---------------------------------------------------------------- host prep

def host_prep_shared(inputs):
    w_ih0 = np.asarray(inputs["w_ih0"])[PERM]
    w_hh0 = np.asarray(inputs["w_hh0"])[PERM]
    b0 = (np.asarray(inputs["b_ih0"]) + np.asarray(inputs["b_hh0"]))[PERM]
    w_ih1 = np.asarray(inputs["w_ih1"])[PERM]
    w_hh1 = np.asarray(inputs["w_hh1"])[PERM]
    b1 = (np.asarray(inputs["b_ih1"]) + np.asarray(inputs["b_hh1"]))[PERM]
    fc_w = np.asarray(inputs["fc_w"])
    fc_b = np.asarray(inputs["fc_b"])

    w0T = np.zeros((512, 2048), np.float32)
    w0T[:500] = w_ih0.T
    w0T[500] = b0
    wh0T = np.ascontiguousarray(w_hh0.T, dtype=np.float32)
    w1xT = np.ascontiguousarray(w_ih1.T, dtype=np.float32)
    wh1T = np.ascontiguousarray(w_hh1.T, dtype=np.float32)
    b1row = np.ascontiguousarray(b1[None, :], dtype=np.float32)
    ones1 = np.ones((1, 128), np.float32)
    fcwT = np.zeros((512, FH), np.float32)
    fcwT[:, :OUT_DIM] = fc_w.T
    fcbT = np.zeros((32, 512), np.float32)
    fcbT[:, :500] = fc_b[:16000].reshape(500, 32).T
    fcbF4 = np.ascontiguousarray(
        np.tile(fc_b[16000:16032].reshape(32, 1), (4, 1)), dtype=np.float32)
    ident = np.ascontiguousarray(np.tile(np.eye(32, dtype=np.float32), (4, 1)))
    import ml_dtypes
    tobf = lambda a: np.ascontiguousarray(a, dtype=ml_dtypes.bfloat16)
    return dict(w0T=tobf(w0T), wh0T=tobf(wh0T), w1xT=tobf(w1xT),
                wh1T=tobf(wh1T), b1row=tobf(b1row), ones1=tobf(ones1),
                fcwT=tobf(fcwT), fcbT=fcbT, fcbF4=fcbF4, identt=ident,
                identb=tobf(ident))


def host_prep_x(x_core):
    """x_core [BL, T, 500] -> xT [512, T*BL], (t, b) free order, ones bias row."""
    T = x_core.shape[1]
    import ml_dtypes
    xT = np.zeros((512, T * BL), np.float32)
    xT[:500] = np.asarray(x_core, np.float32).transpose(2, 1, 0).reshape(500, T * BL)
    xT[500] = 1.0
    return np.ascontiguousarray(xT, dtype=ml_dtypes.bfloat16)


# ---------------------------------------------------------------- bass build

def build_nc(T=T_FULL):
    nc = bacc.Bacc("TRN2")

    xT_d = nc.dram_tensor("xT", [512, T * BL], BF, kind="ExternalInput")
    w0T_d = nc.dram_tensor("w0T", [512, 2048], BF, kind="ExternalInput")
    wh0T_d = nc.dram_tensor("wh0T", [512, 2048], BF, kind="ExternalInput")
    w1xT_d = nc.dram_tensor("w1xT", [512, 2048], BF, kind="ExternalInput")
    wh1T_d = nc.dram_tensor("wh1T", [512, 2048], BF, kind="ExternalInput")
    b1row_d = nc.dram_tensor("b1row", [1, 2048], BF, kind="ExternalInput")
    ones1_d = nc.dram_tensor("ones1", [1, 128], BF, kind="ExternalInput")
    fcwT_d = nc.dram_tensor("fcwT", [512, FH], BF, kind="ExternalInput")
    fcbT_d = nc.dram_tensor("fcbT", [32, 512], FP, kind="ExternalInput")
    fcbF4_d = nc.dram_tensor("fcbF4", [128, 1], FP, kind="ExternalInput")
    identt_d = nc.dram_tensor("identt", [128, 32], FP, kind="ExternalInput")
    identb_d = nc.dram_tensor("identb", [128, 32], BF, kind="ExternalInput")

    sigma_d = nc.dram_tensor("sigma", [BL, NA, NA], F16, kind="ExternalOutput")
    idio_d = nc.dram_tensor("idio_raw", [BL, NA], BF, kind="ExternalOutput")

    def mm(out, lhsT, rhs, tp, **kw):
        nc.tensor.matmul(out, lhsT, rhs,
                         tile_position=tp, skip_group_check=True, **kw)

    def tr(out, in_, identity, tp):
        nc.tensor.matmul(out, in_, identity, is_transpose=True,
                         tile_position=tp, skip_group_check=True)

    with TileContext(nc) as tc:
        with tc.tile_pool(name="persist", bufs=1) as persist:
            ones1_sb = persist.tile([1, 128], BF)
            nc.sync.dma_start(ones1_sb, ones1_d[:, :])
            b1row_sb = persist.tile([1, 2048], BF)
            nc.sync.dma_start(b1row_sb, b1row_d[:, :])
            identt_sb = persist.tile([128, 32], FP)
            nc.sync.dma_start(identt_sb, identt_d[:, :])
            identb_sb = persist.tile([128, 32], BF)
            nc.sync.dma_start(identb_sb, identb_d[:, :])
            fcbT_sb = persist.tile([32, 512], FP)
            nc.sync.dma_start(fcbT_sb, fcbT_d[:, :])
            fcbF4_sb = persist.tile([128, 1], FP)
            nc.sync.dma_start(fcbF4_sb, fcbF4_d[:, :])
            hlast = persist.tile([128, 128], BF)  # final h1T, chunk-major cols
            # prefetched fcw feature tiles (DMA issued after the LSTM weight
            # loads so it doesn't delay the LSTM start)
            fcw_pref = persist.tile([128, 4, N_PREF * 512], BF)

            # ---------------- phase 1: LSTM ----------------
            # Both layers' input projections run once per 4-step BLOCK with a
            # full [128,128] stationary (weights stream 1x per block instead
            # of per step).  Block outputs land in PSUM with rows (m, b)
            # (m = step-in-block); per-(m, j) shift-DMAs re-align them to the
            # per-step (hidden-group, b) gate layout in SBUF (xga).  The
            # per-step evac then adds xga onto the recurrent PSUM gates.
            # Layer 1 consumes blocked h0 projections SKEW steps behind l0.
            SKEW = 5
            with (
                tc.tile_pool(name="wconst", bufs=1) as wconst,
                tc.tile_pool(name="xring", bufs=2) as xring,
                tc.tile_pool(name="state", bufs=2) as state,
                tc.tile_pool(name="xgap", bufs=2) as xgap,
                tc.tile_pool(name="work", bufs=2) as work,
                tc.tile_pool(name="pgates", bufs=8, space="PSUM") as pgates,
            ):
                w0T_sb = wconst.tile([128, 4, 2048], BF)
                nc.sync.dma_start(w0T_sb, w0T_d.rearrange("(ko p) g -> p ko g", p=128))
                wh0T_sb = wconst.tile([128, 4, 2048], BF)
                nc.sync.dma_start(wh0T_sb, wh0T_d.rearrange("(ko p) g -> p ko g", p=128))
                w1x_sb = wconst.tile([128, 4, 2048], BF)
                nc.sync.dma_start(w1x_sb, w1xT_d.rearrange("(ko p) g -> p ko g", p=128))
                wh1_sb = wconst.tile([128, 4, 2048], BF)
                nc.sync.dma_start(wh1_sb, wh1T_d.rearrange("(ko p) g -> p ko g", p=128))

                xch = min(XCHUNK, T)
                n_xchunks = (T + xch - 1) // xch
                n_blocks = T // 4
                x_tiles = {}

                def load_xchunk(ci):
                    if ci >= n_xchunks:
                        return
                    xt = xring.tile([128, 4, xch * BL], BF, tag="xchunk")
                    nc.sync.dma_start(
                        xt,
                        xT_d[:, ci * xch * BL:(ci + 1) * xch * BL]
                        .rearrange("(ko p) tb -> p ko tb", p=128),
                    )
                    x_tiles[ci] = xt

                load_xchunk(0)
                # fcw prefetch rides the idle DMA bandwidth behind the weights
                nc.scalar.dma_start(
                    fcw_pref,
                    fcwT_d[:, :N_PREF * 512].rearrange("(ko p) n -> p ko n",
                                                       p=128),
                )

                def emit_xp_block(which, B, h0blk_src=None):
                    """Blocked input projection for steps [4B, 4B+4).
                    which=0: x @ w0T (bias via xT ones row);
                    which=1: h0 @ w1xT + b1 (bias via 1-row matmul).
                    Returns xga [128=(j,b), 4=m, 512] in SBUF."""
                    xpps = [pgates.tile([128, 512], FP, tag="g",
                                        name=f"xpp{which}_{j}")
                            for j in range(4)]
                    if which == 0:
                        xt = x_tiles[B // 4]
                        rel = B % 4
                        for k in range(4):
                            lhsT = xt[:, k, rel * 128:(rel + 1) * 128]
                            for j in range(4):
                                mm(xpps[j][:, :], lhsT,
                                   w0T_sb[:, k, 512 * j:512 * (j + 1)],
                                   tp=(0, 0), start=(k == 0), stop=(k == 3))
                    else:
                        for j in range(4):
                            mm(xpps[j][:, :], ones1_sb[0:1, :],
                               b1row_sb[0:1, 512 * j:512 * (j + 1)],
                               tp=(0, 0), start=True, stop=False)
                        for k in range(4):
                            lhsT = h0blk_src[:, k, :]
                            for j in range(4):
                                mm(xpps[j][:, :], lhsT,
                                   w1x_sb[:, k, 512 * j:512 * (j + 1)],
                                   tp=(0, 0), start=False, stop=(k == 3))
                    # PSUM -> SBUF block evac (DMA cannot read PSUM), then
                    # partition-shifting DMAs build the per-step layout
                    xgb = xgap.tile([128, 4, 512], BF, tag=f"xgb{which}")
                    for j in range(4):
                        if j % 2 == 0:
                            nc.scalar.copy(xgb[:, j, :], xpps[j][:, :])
                        else:
                            nc.vector.tensor_copy(xgb[:, j, :], xpps[j][:, :])
                    xga = xgap.tile([128, 4, 512], BF, tag=f"xga{which}")
                    for m_ in range(4):
                        for j in range(4):
                            nc.sync.dma_start(
                                xga[32 * j:32 * (j + 1), m_, :],
                                xgb[32 * m_:32 * (m_ + 1), j, :])
                    return xga

                def gates_banks(nm):
                    # one PSUM bank per hidden-slice col group: concurrent
                    # col-tiled matmuls into the SAME bank corrupt on HW.
                    return [pgates.tile([128, 512], FP, tag="g", name=f"{nm}{j}")
                            for j in range(4)]

                def evac(pgs, dst, xga, m_):
                    """dst[32j:+32] = pgs[j][32j:+32] + xga[32j:+32, m_, :].
                    All on DVE: GpSimd is far too slow on 32-partition tiles
                    (only 2 of 8 DSPs engaged) and stalls the nonlinearity."""
                    for j in range(4):
                        s = slice(32 * j, 32 * (j + 1))
                        if pgs is None:
                            if j % 2 == 0:
                                nc.scalar.copy(dst[s, :], xga[s, m_, :])
                            else:
                                nc.vector.tensor_copy(dst[s, :], xga[s, m_, :])
                        else:
                            nc.vector.tensor_add(dst[s, :], pgs[j][s, :],
                                                 xga[s, m_, :])
                    return dst

                def gate_nonlin(ga, cprev, cnew, tag):
                    """ga [128 = (hg, b), 512 = i|f|g|o x128] SBUF -> (H, cnew)."""
                    a = work.tile([128, 512], FP, tag=f"act_{tag}")
                    nc.scalar.activation(a[:, 0:256], ga[:, 0:256], AF.Sigmoid)
                    nc.scalar.activation(a[:, 256:384], ga[:, 256:384], AF.Tanh)
                    nc.scalar.activation(a[:, 384:512], ga[:, 384:512], AF.Sigmoid)
                    t1 = work.tile([128, 128], FP, tag=f"t1_{tag}")
                    nc.vector.tensor_mul(t1, a[:, 0:128], a[:, 256:384])
                    if cprev is None:
                        cn = t1  # c_prev == 0 at t == 0
                    else:
                        t2 = work.tile([128, 128], FP, tag=f"t2_{tag}")
                        nc.vector.tensor_mul(t2, a[:, 128:256], cprev)
                        cn = cnew
                        nc.vector.tensor_add(cn, t1, t2)
                    tcn = work.tile([128, 128], FP, tag=f"tc_{tag}")
                    nc.scalar.activation(tcn, cn, AF.Tanh)
                    hh = work.tile([128, 128], FP, tag=f"h_{tag}")
                    nc.vector.tensor_mul(hh, a[:, 384:512], tcn)
                    return hh, cn

                def transpose_cols(hh, dst_cols, tag):
                    """hh [128=(hg,b),128] -> bf16 transposed into dst_cols[k]
                    [128, 32] slices; each 32-col transpose gets its OWN psum
                    bank (concurrent row-tiled transposes corrupt on HW)."""
                    for k in range(4):
                        ptk = pgates.tile([128, 32], FP, tag="g",
                                          name=f"pt{tag}{k}")
                        tr(ptk, hh[32 * k:32 * (k + 1), :],
                           identt_sb[32 * k:32 * (k + 1), :], (32 * k, 0))
                        nc.vector.tensor_copy(dst_cols[k], ptk)

                c0 = c1 = None
                h1T = None
                h0blk_cur = h0blk_prev = None
                xga0 = {}
                xga1 = {}
                xga0[0] = emit_xp_block(0, 0)
                for s in range(T + SKEW):
                    # ---- all matmuls first: the PE queue is in-order, so
                    # transposes (which wait on ACT/DVE) must not sit ahead
                    # of independent rec matmuls
                    pgs0 = pgs1 = None
                    if s < T:
                        m, B = s % 4, s // 4
                        if m == 0:
                            h0blk_prev = h0blk_cur
                            h0blk_cur = state.tile([128, 4, 128], BF,
                                                   tag="h0blk")
                            if B % 4 == 2:
                                # next x chunk, 5 supersteps before the first
                                # xp block emission that reads it
                                load_xchunk(B // 4 + 1)
                        if s > 0:
                            pgs0 = gates_banks("g0_")
                            src = h0blk_cur if m > 0 else h0blk_prev
                            mp = (s - 1) % 4
                            for k in range(4):
                                lhsT = src[:, k, 32 * mp:32 * (mp + 1)]
                                for j in range(4):
                                    mm(pgs0[j][32 * j:32 * (j + 1), :], lhsT,
                                       wh0T_sb[:, k, 512 * j:512 * (j + 1)],
                                       tp=(0, 32 * j),
                                       start=(k == 0), stop=(k == 3))
                    if s >= SKEW:
                        t1_ = s - SKEW
                        m1, B1 = t1_ % 4, t1_ // 4
                        if t1_ > 0:
                            pgs1 = gates_banks("g1_")
                            for k in range(4):
                                lhsT = h1T[:, 32 * k:32 * (k + 1)]
                                for j in range(4):
                                    mm(pgs1[j][32 * j:32 * (j + 1), :], lhsT,
                                       wh1_sb[:, k, 512 * j:512 * (j + 1)],
                                       tp=(0, 32 * j),
                                       start=(k == 0), stop=(k == 3))

                    # ---- xp0 block for an upcoming block: no deps on this
                    # superstep's posts, fills the PE while ACT/DVE work
                    if s < T and m == 1 and B + 1 < n_blocks:
                        xga0[B + 1] = emit_xp_block(0, B + 1)

                    # ---- layer-0 post-processing
                    if s < T:
                        a0 = work.tile([128, 512], FP, tag="ga_l0")
                        evac(pgs0, a0, xga0[B], m)
                        c0n = None if c0 is None else state.tile(
                            [128, 128], FP, tag="c0")
                        h0, c0 = gate_nonlin(a0, c0, c0n, "l0")
                        transpose_cols(
                            h0, [h0blk_cur[:, k, 32 * m:32 * (m + 1)]
                                 for k in range(4)], "l0")
                        if m == 3:
                            xga1[B] = emit_xp_block(1, B, h0blk_cur)

                    # ---- layer-1 post-processing
                    if s >= SKEW:
                        a1 = work.tile([128, 512], FP, tag="ga_l1")
                        evac(pgs1, a1, xga1[B1], m1)
                        c1n = None if c1 is None else state.tile(
                            [128, 128], FP, tag="c1")
                        h1, c1 = gate_nonlin(a1, c1, c1n, "l1")
                        h1T_n = state.tile([128, 128], BF, tag="ht_l1")
                        transpose_cols(
                            h1, [h1T_n[:, 32 * k:32 * (k + 1)]
                                 for k in range(4)], "l1")
                        h1T = h1T_n

                nc.vector.tensor_copy(hlast, h1T)

            # ---------------- phase 2: FC + Lambda layout + Sigma ----------------
            # LT holds loadings as [factor, b, asset] replicated on 4
            # partition groups after FC, so sigma matmuls can rotate PE row
            # positions (hides LDWEIGHTS) and stream contiguous rows.
            with tc.tile_pool(name="lt", bufs=1) as ltp:
                LT = ltp.tile([128, 32, 500], BF)   # [factor(x4), b, asset]
                F4 = ltp.tile([128, 32], FP)        # exp(fvar+bias) x4 groups

                with (
                    tc.tile_pool(name="fcw", bufs=3) as fcwp,
                    tc.tile_pool(name="rawp", bufs=3) as rawp,
                    tc.tile_pool(name="pfc", bufs=4, space="PSUM") as pfcp,
                    tc.tile_pool(name="plt", bufs=2, space="PSUM") as pltp,
                ):
                    n_quads = (N_FTILES + 3) // 4      # 9 (last quad: 1 tile)
                    for q in range(n_quads):
                        rr = range(4) if q < 8 else range(1)
                        raw_t = rawp.tile([128, 512], BF, tag="raw")
                        srcs, pfcs = {}, {}
                        for r in rr:
                            jj = 4 * q + r
                            if jj < N_PREF:
                                srcs[r] = fcw_pref[:, :, jj * 512:(jj + 1) * 512]
                            else:
                                fcw_t = fcwp.tile([128, 4, 512], BF, tag="fcw")
                                nc.sync.dma_start(
                                    fcw_t,
                                    fcwT_d[:, jj * 512:(jj + 1) * 512]
                                    .rearrange("(ko p) n -> p ko n", p=128),
                                )
                                srcs[r] = fcw_t
                            pfcs[r] = pfcp.tile([128, 512], FP, tag="pfc",
                                                name=f"pfc{r}")
                        # k outer / r inner: consecutive matmuls sit at
                        # different PE col positions so LDWEIGHTS overlaps
                        for k in range(4):
                            for r in rr:
                                mm(pfcs[r][32 * r:32 * (r + 1), :],
                                   hlast[:, 32 * k:32 * (k + 1)],
                                   srcs[r][:, k, :],
                                   tp=(0, 32 * r),
                                   start=(k == 0), stop=(k == 3))
                        for r in rr:
                            s = slice(32 * r, 32 * (r + 1))
                            if r % 2 == 0:
                                nc.scalar.copy(raw_t[s, :], pfcs[r][s, :])
                            else:
                                nc.vector.tensor_copy(raw_t[s, :], pfcs[r][s, :])

                        # Lambda blocks inside this quad -> transpose into LT
                        for r in rr:
                            jj = 4 * q + r
                            base_feat = jj * 512
                            nblk = 16 if jj < 31 else (4 if jj == 31 else 0)
                            for g in range(0, nblk, 4):
                                pt = pltp.tile([32, 128], BF, tag="plt")
                                for a in range(4):
                                    blk = g + a
                                    tr(pt[:, 32 * a:32 * (a + 1)],
                                       raw_t[32 * r:32 * (r + 1),
                                             32 * blk:32 * (blk + 1)],
                                       identb_sb[32 * r:32 * (r + 1), :],
                                       (32 * r, 0))
                                a0 = (base_feat + 32 * g) // 32  # asset index
                                nc.vector.tensor_tensor(
                                    LT[0:32, :, a0:a0 + 4],
                                    pt.rearrange("f (a b) -> f b a", a=4),
                                    fcbT_sb[:, None, a0:a0 + 4]
                                    .to_broadcast([32, 32, 4]),
                                    mybir.AluOpType.add,
                                )
                            if jj == 31:
                                # fvar feats 16000:16032 = cols 128:160 (r==3)
                                # transpose 4x at rotated col positions to
                                # build all four partition-group replicas
                                for rp in range(4):
                                    ptf = pltp.tile([128, 32], BF, tag="pltf")
                                    tr(ptf[32 * rp:32 * (rp + 1), 0:32],
                                       raw_t[96:128, 128:160],
                                       identb_sb[96:128, :], (96, 32 * rp))
                                    nc.scalar.activation(
                                        F4[32 * rp:32 * (rp + 1), :],
                                        ptf[32 * rp:32 * (rp + 1), 0:32],
                                        AF.Exp,
                                        bias=fcbF4_sb[32 * rp:32 * (rp + 1),
                                                      0:1])
                                # idio part 1: feats 16032:16384 = cols 160:512
                                nc.sync.dma_start(idio_d[:, 0:352],
                                                  raw_t[96:128, 160:512])
                            if jj == 32:
                                # idio part 2: feats 16384:16532 = cols 0:148
                                nc.sync.dma_start(idio_d[:, 352:500],
                                                  raw_t[0:32, 0:148])

                # replicate loadings to partition groups 1..3 (parallel queues)
                for rp in range(1, 4):
                    eng = [nc.scalar, nc.gpsimd, nc.sync][rp - 1]
                    eng.dma_start(LT[32 * rp:32 * (rp + 1), :, :],
                                  LT[0:32, :, :])

                # Sigma: 4 m-tiles of 125 rows per sample, staged fp16, one
                # fused DMA per pair; samples rotate PE row positions so
                # LDWEIGHTS of one overlaps the matmul of the other.
                with (
                    tc.tile_pool(name="sigw", bufs=2) as sigw,
                    tc.tile_pool(name="psig", bufs=2, space="PSUM") as psigp,
                ):
                    for b0 in range(0, BL, 2):
                        st2 = sigw.tile([128, 2, 4, 500], F16, tag="sigstage")
                        gts = []
                        for bb in range(2):
                            b = b0 + bb
                            rp = b % 4
                            sp = slice(32 * rp, 32 * (rp + 1))
                            gt = sigw.tile([128, 512], BF, tag="gt")
                            nc.vector.tensor_scalar_mul(
                                gt[sp, 0:500], LT[sp, b, :], F4[sp, b:b + 1])
                            gts.append((gt, sp, rp, b))
                        pss = [psigp.tile([128, 2048], FP, tag="psig",
                                          name=f"psig{i}")
                               for i in range(2)]
                        for mt in range(4):
                            for bb in range(2):
                                gt, sp, rp, b = gts[bb]
                                mm(pss[bb][:125, 512 * mt:512 * mt + 500],
                                   gt[sp, 125 * mt:125 * mt + 125],
                                   LT[sp, b, :],
                                   tp=(32 * rp, 0), start=True, stop=True)
                        for bb in range(2):
                            src = pss[bb].rearrange("p (m n) -> p m n", m=4)
                            if bb % 2 == 0:
                                nc.scalar.copy(st2[:125, bb, :, :],
                                               src[:125, :, 0:500])
                            else:
                                nc.vector.tensor_copy(st2[:125, bb, :, :],
                                                      src[:125, :, 0:500])
                        eng = [nc.sync, nc.scalar,
                               nc.gpsimd][(b0 // 2) % 3]
                        eng.dma_start(
                            sigma_d[b0:b0 + 2].rearrange(
                                "b (m p) n -> p b m n", p=125),
                            st2[:125])

    nc.compile()
    return nc


# ---------------------------------------------------------------- entry point

def kernel(**inputs):
    from concourse.bass_utils import run_bass_kernel_spmd

    prep = host_prep_shared(inputs)
    x = np.asarray(inputs["x"], np.float32)
    in_maps = []
    for core in range(NCORES):
        m = dict(prep)
        m["xT"] = host_prep_x(x[core * BL:(core + 1) * BL])
        in_maps.append(m)

    nc = build_nc()
    res = run_bass_kernel_spmd(nc, in_maps, list(range(NCORES)))
    results = res.results

    fcb_idio = np.asarray(inputs["fc_b"], np.float32)[16032:16532]
    idx = np.arange(NA)
    out = np.empty((B_FULL, NA, NA), np.float32)
    for core in range(NCORES):
        sigma = np.asarray(results[core]["sigma"]).astype(np.float32)
        idio = np.exp(np.asarray(results[core]["idio_raw"], np.float32)
                      + fcb_idio[None, :])
        sigma[:, idx, idx] += idio
        out[core * BL:(core + 1) * BL] = sigma
    return out



# revision 50
# speedup vs baseline: 1.2988x; 1.0813x over previous
"""Trainium2 Bass kernel for nn_FactorCovModel.

Model: 2-layer LSTM (H=512) over [B=256, T=64, D=500], last hidden ->
FC [512 -> 16532] -> Sigma = Lambda diag(exp(fv)) Lambda^T + diag(exp(idio)),
output [256, 500, 500].

Sharding: pure data parallel over batch, 32 samples/core on 8 cores.

Per-core device algorithm (matmul operands bf16, fp32 PSUM accumulation):
  - Weight gate axis host-permuted so PSUM col group hg holds hidden slice
    hg of ALL FOUR gates: PSUM [128 = (hg, batch), 512 = i|f|g|o x 128].
    Every ACT/DVE op is then full-128-partition and partition-aligned.
  - LSTM gates are computed column-tiled: stationary = hT chunk [128, 32],
    4 hidden-slice groups run concurrently at tile positions (0, 32j).
  - x-projection (xg0) matmuls accumulate into the same PSUM tile two
    steps ahead; recurrent matmuls then add onto it (start=False).
  - Layer-1 consumes h0T[t] directly (fused input projection, contraction
    [h0T; h1T] = 1024) plus a bias inject via a stacked-identity stationary.
  - FC runs col-packed (4 feature tiles of 512 per PSUM tile), then Lambda
    is re-laid-out via 500 PE transposes of [32, 32] blocks into
    LT [32 factors, 500 assets, 32 batch]; fvar gets exp via ACT.
  - Sigma_b = (LT_b * f_b)^T @ LT_b per sample, 4 m-tiles of 128.
  - idio raw rows go back to the host, which applies bias+exp and adds the
    diagonal (avoids diagonal APs on device).
"""

import os
import sys

sys.path.insert(0, "/opt/trn_rl_repo")

import numpy as np

import concourse.bass as bass
import concourse.mybir as mybir
from concourse import bacc
from concourse.tile import TileContext

FP = mybir.dt.float32
BF = mybir.dt.bfloat16
F16 = mybir.dt.float16
AF = mybir.ActivationFunctionType

B_FULL, T_FULL, D_IN, H = 256, 64, 500, 512
NCORES = 8
BL = B_FULL // NCORES            # 32 samples per core
NA, NF = 500, 32                 # assets, factors
OUT_DIM = NA * NF + NF + NA      # 16532
NTILE = 512                      # FC feature tile
N_FTILES = 33                    # ceil(16532/512) -> features padded to 16896
FH = N_FTILES * NTILE            # 16896
XCHUNK = 16                      # time steps per streamed xT chunk
N_PREF = 8                       # fcw feature tiles prefetched during LSTM

# gate-axis permutation: new col (hg, gate, hl) = 512*hg + 128*gate + hl maps to
# old row gate*512 + 128*hg + hl (torch gate order [i, f, g, o]).  With this
# layout, PSUM col group hg holds ALL FOUR gates of hidden slice hg along the
# free dim, so every ACT/DVE op is full-128-partition and partition-aligned.
PERM = np.array([gate * 512 + 128 * hg + hl
                 for hg in range(4) for gate in range(4) for hl in range(128)])


# ---------------------------------------------------------------- host prep

def host_prep_shared(inputs):
    w_ih0 = np.asarray(inputs["w_ih0"])[PERM]
    w_hh0 = np.asarray(inputs["w_hh0"])[PERM]
    b0 = (np.asarray(inputs["b_ih0"]) + np.asarray(inputs["b_hh0"]))[PERM]
    w_ih1 = np.asarray(inputs["w_ih1"])[PERM]
    w_hh1 = np.asarray(inputs["w_hh1"])[PERM]
    b1 = (np.asarray(inputs["b_ih1"]) + np.asarray(inputs["b_hh1"]))[PERM]
    fc_w = np.asarray(inputs["fc_w"])
    fc_b = np.asarray(inputs["fc_b"])

    w0T = np.zeros((512, 2048), np.float32)
    w0T[:500] = w_ih0.T
    w0T[500] = b0
    wh0T = np.ascontiguousarray(w_hh0.T, dtype=np.float32)
    w1xT = np.ascontiguousarray(w_ih1.T, dtype=np.float32)
    wh1T = np.ascontiguousarray(w_hh1.T, dtype=np.float32)
    b1row = np.ascontiguousarray(b1[None, :], dtype=np.float32)
    # scale the g-gate preacts by 2 so the device can use a single sigmoid
    # activation for all gates: tanh(x) = 2*sigmoid(2x) - 1
    gcols = np.zeros(2048, bool)
    for hg in range(4):
        gcols[512 * hg + 256:512 * hg + 384] = True
    for wmat in (w0T, wh0T, w1xT, wh1T, b1row):
        wmat[:, gcols] *= 2.0
    ones1 = np.ones((1, 128), np.float32)
    fcwT = np.zeros((512, FH), np.float32)
    fcwT[:, :OUT_DIM] = fc_w.T
    fcbT = np.zeros((32, 512), np.float32)
    fcbT[:, :500] = fc_b[:16000].reshape(500, 32).T
    fcbF4 = np.ascontiguousarray(
        np.tile(fc_b[16000:16032].reshape(32, 1), (4, 1)), dtype=np.float32)
    ident = np.ascontiguousarray(np.tile(np.eye(32, dtype=np.float32), (4, 1)))
    import ml_dtypes
    tobf = lambda a: np.ascontiguousarray(a, dtype=ml_dtypes.bfloat16)
    return dict(w0T=tobf(w0T), wh0T=tobf(wh0T), w1xT=tobf(w1xT),
                wh1T=tobf(wh1T), b1row=tobf(b1row), ones1=tobf(ones1),
                fcwT=tobf(fcwT), fcbT=fcbT, fcbF4=fcbF4, identt=ident,
                identb=tobf(ident))


def host_prep_x(x_core):
    """x_core [BL, T, 500] -> xT [512, T*BL], (t, b) free order, ones bias row."""
    T = x_core.shape[1]
    import ml_dtypes
    xT = np.zeros((512, T * BL), np.float32)
    xT[:500] = np.asarray(x_core, np.float32).transpose(2, 1, 0).reshape(500, T * BL)
    xT[500] = 1.0
    return np.ascontiguousarray(xT, dtype=ml_dtypes.bfloat16)


# ---------------------------------------------------------------- bass build

def build_nc(T=T_FULL):
    nc = bacc.Bacc("TRN2")

    xT_d = nc.dram_tensor("xT", [512, T * BL], BF, kind="ExternalInput")
    w0T_d = nc.dram_tensor("w0T", [512, 2048], BF, kind="ExternalInput")
    wh0T_d = nc.dram_tensor("wh0T", [512, 2048], BF, kind="ExternalInput")
    w1xT_d = nc.dram_tensor("w1xT", [512, 2048], BF, kind="ExternalInput")
    wh1T_d = nc.dram_tensor("wh1T", [512, 2048], BF, kind="ExternalInput")
    b1row_d = nc.dram_tensor("b1row", [1, 2048], BF, kind="ExternalInput")
    ones1_d = nc.dram_tensor("ones1", [1, 128], BF, kind="ExternalInput")
    fcwT_d = nc.dram_tensor("fcwT", [512, FH], BF, kind="ExternalInput")
    fcbT_d = nc.dram_tensor("fcbT", [32, 512], FP, kind="ExternalInput")
    fcbF4_d = nc.dram_tensor("fcbF4", [128, 1], FP, kind="ExternalInput")
    identt_d = nc.dram_tensor("identt", [128, 32], FP, kind="ExternalInput")
    identb_d = nc.dram_tensor("identb", [128, 32], BF, kind="ExternalInput")

    sigma_d = nc.dram_tensor("sigma", [BL, NA, NA], F16, kind="ExternalOutput")
    idio_d = nc.dram_tensor("idio_raw", [BL, NA], BF, kind="ExternalOutput")

    def mm(out, lhsT, rhs, tp, **kw):
        nc.tensor.matmul(out, lhsT, rhs,
                         tile_position=tp, skip_group_check=True, **kw)

    def tr(out, in_, identity, tp):
        nc.tensor.matmul(out, in_, identity, is_transpose=True,
                         tile_position=tp, skip_group_check=True)

    with TileContext(nc) as tc:
        with tc.tile_pool(name="persist", bufs=1) as persist:
            ones1_sb = persist.tile([1, 128], BF)
            nc.sync.dma_start(ones1_sb, ones1_d[:, :])
            b1row_sb = persist.tile([1, 2048], BF)
            nc.sync.dma_start(b1row_sb, b1row_d[:, :])
            identt_sb = persist.tile([128, 32], FP)
            nc.sync.dma_start(identt_sb, identt_d[:, :])
            identb_sb = persist.tile([128, 32], BF)
            nc.sync.dma_start(identb_sb, identb_d[:, :])
            fcbT_sb = persist.tile([32, 512], FP)
            nc.sync.dma_start(fcbT_sb, fcbT_d[:, :])
            fcbF4_sb = persist.tile([128, 1], FP)
            nc.sync.dma_start(fcbF4_sb, fcbF4_d[:, :])
            hlast = persist.tile([128, 128], BF)  # final h1T, chunk-major cols
            # prefetched fcw feature tiles (DMA issued after the LSTM weight
            # loads so it doesn't delay the LSTM start)
            fcw_pref = persist.tile([128, 4, N_PREF * 512], BF)

            # ---------------- phase 1: LSTM ----------------
            # Both layers' input projections run once per 4-step BLOCK with a
            # full [128,128] stationary (weights stream 1x per block instead
            # of per step).  Block outputs land in PSUM with rows (m, b)
            # (m = step-in-block); per-(m, j) shift-DMAs re-align them to the
            # per-step (hidden-group, b) gate layout in SBUF (xga).  The
            # per-step evac then adds xga onto the recurrent PSUM gates.
            # Layer 1 consumes blocked h0 projections SKEW steps behind l0.
            SKEW = 5
            with (
                tc.tile_pool(name="wconst", bufs=1) as wconst,
                tc.tile_pool(name="xring", bufs=2) as xring,
                tc.tile_pool(name="state", bufs=2) as state,
                tc.tile_pool(name="xgap", bufs=2) as xgap,
                tc.tile_pool(name="work", bufs=2) as work,
                tc.tile_pool(name="pgates", bufs=8, space="PSUM") as pgates,
            ):
                w0T_sb = wconst.tile([128, 4, 2048], BF)
                nc.sync.dma_start(w0T_sb, w0T_d.rearrange("(ko p) g -> p ko g", p=128))
                wh0T_sb = wconst.tile([128, 4, 2048], BF)
                nc.sync.dma_start(wh0T_sb, wh0T_d.rearrange("(ko p) g -> p ko g", p=128))
                w1x_sb = wconst.tile([128, 4, 2048], BF)
                nc.sync.dma_start(w1x_sb, w1xT_d.rearrange("(ko p) g -> p ko g", p=128))
                wh1_sb = wconst.tile([128, 4, 2048], BF)
                nc.sync.dma_start(wh1_sb, wh1T_d.rearrange("(ko p) g -> p ko g", p=128))

                xch = min(XCHUNK, T)
                n_xchunks = (T + xch - 1) // xch
                n_blocks = T // 4
                x_tiles = {}

                def load_xchunk(ci):
                    if ci >= n_xchunks:
                        return
                    xt = xring.tile([128, 4, xch * BL], BF, tag="xchunk")
                    nc.sync.dma_start(
                        xt,
                        xT_d[:, ci * xch * BL:(ci + 1) * xch * BL]
                        .rearrange("(ko p) tb -> p ko tb", p=128),
                    )
                    x_tiles[ci] = xt

                load_xchunk(0)
                # fcw prefetch rides the idle DMA bandwidth behind the weights
                nc.scalar.dma_start(
                    fcw_pref,
                    fcwT_d[:, :N_PREF * 512].rearrange("(ko p) n -> p ko n",
                                                       p=128),
                )

                def emit_xp_block(which, B, h0blk_src=None):
                    """Blocked input projection for steps [4B, 4B+4).
                    which=0: x @ w0T (bias via xT ones row);
                    which=1: h0 @ w1xT + b1 (bias via 1-row matmul).
                    Returns xga [128=(j,b), 4=m, 512] in SBUF."""
                    xpps = [pgates.tile([128, 512], FP, tag="g",
                                        name=f"xpp{which}_{j}")
                            for j in range(4)]
                    if which == 0:
                        xt = x_tiles[B // 4]
                        rel = B % 4
                        for k in range(4):
                            lhsT = xt[:, k, rel * 128:(rel + 1) * 128]
                            for j in range(4):
                                mm(xpps[j][:, :], lhsT,
                                   w0T_sb[:, k, 512 * j:512 * (j + 1)],
                                   tp=(0, 0), start=(k == 0), stop=(k == 3))
                    else:
                        for j in range(4):
                            mm(xpps[j][:, :], ones1_sb[0:1, :],
                               b1row_sb[0:1, 512 * j:512 * (j + 1)],
                               tp=(0, 0), start=True, stop=False)
                        for k in range(4):
                            lhsT = h0blk_src[:, k, :]
                            for j in range(4):
                                mm(xpps[j][:, :], lhsT,
                                   w1x_sb[:, k, 512 * j:512 * (j + 1)],
                                   tp=(0, 0), start=False, stop=(k == 3))
                    # PSUM -> SBUF block evac; the per-step partition shift
                    # happens via identity matmuls into the rec gate banks
                    xgb = xgap.tile([128, 4, 512], BF, tag=f"xgb{which}")
                    for j in range(4):
                        if j % 2 == 0:
                            nc.scalar.copy(xgb[:, j, :], xpps[j][:, :])
                        else:
                            nc.vector.tensor_copy(xgb[:, j, :], xpps[j][:, :])
                    return xgb

                def gates_banks(nm):
                    # one PSUM bank per hidden-slice col group: concurrent
                    # col-tiled matmuls into the SAME bank corrupt on HW.
                    return [pgates.tile([128, 512], FP, tag="g", name=f"{nm}{j}")
                            for j in range(4)]

                def evac(pgs, dst):
                    """Copy group j of pgs -> dst[32j:32j+32], 2 ACT / 2 DVE
                    so the gate nonlinearity isn't gated on one engine."""
                    for j in range(4):
                        s = slice(32 * j, 32 * (j + 1))
                        if j % 2 == 0:
                            nc.scalar.copy(dst[s, :], pgs[j][s, :])
                        else:
                            nc.vector.tensor_copy(dst[s, :], pgs[j][s, :])
                    return dst

                def shift_mms(pgs, xgb, m_, first):
                    """Inject the blocked input projection for step-in-block
                    m_ into the gate banks: identity matmul moves xgb rows
                    (m_, b) -> bank j rows (j, b).  Runs at PE positions
                    (32*m_, 32*j), opening each bank's accumulation group."""
                    for j in range(4):
                        mm(pgs[j][32 * j:32 * (j + 1), :],
                           identb_sb[32 * m_:32 * (m_ + 1), :],
                           xgb[32 * m_:32 * (m_ + 1), j, :],
                           tp=(32 * m_, 32 * j), start=True, stop=first)

                def gate_nonlin(ga, cprev, cnew, tag):
                    """ga [128 = (hg, b), 512 = i|f|g|o x128] SBUF -> (H, cnew).
                    g-gate preacts are host-scaled by 2, so ONE sigmoid covers
                    all four gates (tanh(x) = 2*sigmoid(2x) - 1); the fixup
                    rides DVE scalar_tensor_tensor ops."""
                    a = work.tile([128, 512], FP, tag=f"act_{tag}")
                    nc.scalar.activation(a, ga, AF.Sigmoid)
                    # t1h = (sig(2g) - 0.5) * i = (i * tanh(g)) / 2
                    t1h = work.tile([128, 128], FP, tag=f"t1_{tag}")
                    nc.vector.scalar_tensor_tensor(
                        t1h, a[:, 256:384], 0.5, a[:, 0:128],
                        mybir.AluOpType.subtract, mybir.AluOpType.mult)
                    cn = cnew
                    if cprev is None:
                        nc.vector.tensor_scalar_mul(cn, t1h, 2.0)
                    else:
                        t2 = work.tile([128, 128], FP, tag=f"t2_{tag}")
                        nc.vector.tensor_mul(t2, a[:, 128:256], cprev)
                        nc.vector.scalar_tensor_tensor(
                            cn, t1h, 2.0, t2,
                            mybir.AluOpType.mult, mybir.AluOpType.add)
                    tcn = work.tile([128, 128], FP, tag=f"tc_{tag}")
                    nc.scalar.activation(tcn, cn, AF.Tanh)
                    hh = work.tile([128, 128], FP, tag=f"h_{tag}")
                    nc.vector.tensor_mul(hh, a[:, 384:512], tcn)
                    return hh, cn

                def transpose_cols(hh, dst_cols, tag):
                    """hh [128=(hg,b),128] -> bf16 transposed into dst_cols[k]
                    [128, 32] slices; each 32-col transpose gets its OWN psum
                    bank (concurrent row-tiled transposes corrupt on HW)."""
                    for k in range(4):
                        ptk = pgates.tile([128, 32], FP, tag="g",
                                          name=f"pt{tag}{k}")
                        tr(ptk, hh[32 * k:32 * (k + 1), :],
                           identt_sb[32 * k:32 * (k + 1), :], (32 * k, 0))
                        nc.vector.tensor_copy(dst_cols[k], ptk)

                c0 = c1 = None
                h1T = None
                h0blk_cur = h0blk_prev = None
                xga0 = {}
                xga1 = {}
                xga0[0] = emit_xp_block(0, 0)
                for s in range(T + SKEW):
                    # ---- all matmuls first: the PE queue is in-order, so
                    # transposes (which wait on ACT/DVE) must not sit ahead
                    # of independent rec matmuls
                    pgs0 = pgs1 = None
                    if s < T:
                        m, B = s % 4, s // 4
                        if m == 0:
                            h0blk_prev = h0blk_cur
                            h0blk_cur = state.tile([128, 4, 128], BF,
                                                   tag="h0blk")
                            if B % 4 == 2:
                                # next x chunk, 5 supersteps before the first
                                # xp block emission that reads it
                                load_xchunk(B // 4 + 1)
                        pgs0 = gates_banks("g0_")
                        shift_mms(pgs0, xga0[B], m, first=(s == 0))
                        if s > 0:
                            src = h0blk_cur if m > 0 else h0blk_prev
                            mp = (s - 1) % 4
                            for k in range(4):
                                lhsT = src[:, k, 32 * mp:32 * (mp + 1)]
                                for j in range(4):
                                    mm(pgs0[j][32 * j:32 * (j + 1), :], lhsT,
                                       wh0T_sb[:, k, 512 * j:512 * (j + 1)],
                                       tp=(0, 32 * j),
                                       start=False, stop=(k == 3))
                    if s >= SKEW:
                        t1_ = s - SKEW
                        m1, B1 = t1_ % 4, t1_ // 4
                        pgs1 = gates_banks("g1_")
                        shift_mms(pgs1, xga1[B1], m1, first=(t1_ == 0))
                        if t1_ > 0:
                            for k in range(4):
                                lhsT = h1T[:, 32 * k:32 * (k + 1)]
                                for j in range(4):
                                    mm(pgs1[j][32 * j:32 * (j + 1), :], lhsT,
                                       wh1_sb[:, k, 512 * j:512 * (j + 1)],
                                       tp=(0, 32 * j),
                                       start=False, stop=(k == 3))

                    # ---- xp0 block for an upcoming block: no deps on this
                    # superstep's posts, fills the PE while ACT/DVE work
                    if s < T and m == 1 and B + 1 < n_blocks:
                        xga0[B + 1] = emit_xp_block(0, B + 1)

                    # ---- layer-0 post-processing
                    if s < T:
                        a0 = work.tile([128, 512], FP, tag="ga_l0")
                        evac(pgs0, a0)
                        c0n = state.tile([128, 128], FP, tag="c0")
                        h0, c0 = gate_nonlin(a0, c0, c0n, "l0")
                        transpose_cols(
                            h0, [h0blk_cur[:, k, 32 * m:32 * (m + 1)]
                                 for k in range(4)], "l0")
                        if m == 3:
                            xga1[B] = emit_xp_block(1, B, h0blk_cur)

                    # ---- layer-1 post-processing
                    if s >= SKEW:
                        a1 = work.tile([128, 512], FP, tag="ga_l1")
                        evac(pgs1, a1)
                        c1n = state.tile([128, 128], FP, tag="c1")
                        h1, c1 = gate_nonlin(a1, c1, c1n, "l1")
                        h1T_n = state.tile([128, 128], BF, tag="ht_l1")
                        transpose_cols(
                            h1, [h1T_n[:, 32 * k:32 * (k + 1)]
                                 for k in range(4)], "l1")
                        h1T = h1T_n

                nc.vector.tensor_copy(hlast, h1T)

            # ---------------- phase 2: FC + Lambda layout + Sigma ----------------
            # LT holds loadings as [factor, b, asset] replicated on 4
            # partition groups after FC, so sigma matmuls can rotate PE row
            # positions (hides LDWEIGHTS) and stream contiguous rows.
            with tc.tile_pool(name="lt", bufs=1) as ltp:
                LT = ltp.tile([128, 32, 500], BF)   # [factor(x4), b, asset]
                F4 = ltp.tile([128, 32], FP)        # exp(fvar+bias) x4 groups

                with (
                    tc.tile_pool(name="fcw", bufs=3) as fcwp,
                    tc.tile_pool(name="rawp", bufs=3) as rawp,
                    tc.tile_pool(name="pfc", bufs=4, space="PSUM") as pfcp,
                    tc.tile_pool(name="plt", bufs=4, space="PSUM") as pltp,
                ):
                    n_quads = (N_FTILES + 3) // 4      # 9 (last quad: 1 tile)
                    for q in range(n_quads):
                        rr = range(4) if q < 8 else range(1)
                        raw_t = rawp.tile([128, 512], BF, tag="raw")
                        srcs, pfcs = {}, {}
                        for r in rr:
                            jj = 4 * q + r
                            if jj < N_PREF:
                                srcs[r] = fcw_pref[:, :, jj * 512:(jj + 1) * 512]
                            else:
                                fcw_t = fcwp.tile([128, 4, 512], BF, tag="fcw")
                                nc.sync.dma_start(
                                    fcw_t,
                                    fcwT_d[:, jj * 512:(jj + 1) * 512]
                                    .rearrange("(ko p) n -> p ko n", p=128),
                                )
                                srcs[r] = fcw_t
                            pfcs[r] = pfcp.tile([128, 512], FP, tag="pfc",
                                                name=f"pfc{r}")
                        # k outer / r inner: consecutive matmuls sit at
                        # different PE col positions so LDWEIGHTS overlaps
                        for k in range(4):
                            for r in rr:
                                mm(pfcs[r][32 * r:32 * (r + 1), :],
                                   hlast[:, 32 * k:32 * (k + 1)],
                                   srcs[r][:, k, :],
                                   tp=(0, 32 * r),
                                   start=(k == 0), stop=(k == 3))
                        for r in rr:
                            s = slice(32 * r, 32 * (r + 1))
                            if r % 2 == 0:
                                nc.scalar.copy(raw_t[s, :], pfcs[r][s, :])
                            else:
                                nc.vector.tensor_copy(raw_t[s, :], pfcs[r][s, :])

                        # Lambda blocks -> transpose into LT.  g outer / r
                        # inner so consecutive transposes rotate PE row
                        # positions (hides the per-transpose weight load).
                        for g in range(0, 16, 4):
                            pts = {}
                            for r in rr:
                                jj = 4 * q + r
                                nblk = 16 if jj < 31 else (4 if jj == 31 else 0)
                                if g < nblk:
                                    pts[r] = pltp.tile([32, 128], BF,
                                                       tag="plt",
                                                       name=f"plt{r}")
                            for a in range(4):
                                for r, pt in pts.items():
                                    blk = g + a
                                    tr(pt[:, 32 * a:32 * (a + 1)],
                                       raw_t[32 * r:32 * (r + 1),
                                             32 * blk:32 * (blk + 1)],
                                       identb_sb[32 * r:32 * (r + 1), :],
                                       (32 * r, 0))
                            for r, pt in pts.items():
                                a0 = ((4 * q + r) * 512 + 32 * g) // 32
                                nc.vector.tensor_tensor(
                                    LT[0:32, :, a0:a0 + 4],
                                    pt.rearrange("f (a b) -> f b a", a=4),
                                    fcbT_sb[:, None, a0:a0 + 4]
                                    .to_broadcast([32, 32, 4]),
                                    mybir.AluOpType.add,
                                )
                        if q == 7:
                            # fvar feats 16000:16032 = raw cols 128:160 (r==3)
                            ptf_full = pltp.tile([32, 128], BF, tag="plt",
                                                 name="pltf")
                            ptf = ptf_full[:, 0:32]
                            tr(ptf, raw_t[96:128, 128:160],
                               identb_sb[96:128, :], (96, 0))
                            nc.scalar.activation(F4[0:32, :], ptf, AF.Exp,
                                                 bias=fcbF4_sb[0:32, 0:1])
                            # idio part 1: feats 16032:16384 = cols 160:512
                            nc.sync.dma_start(idio_d[:, 0:352],
                                              raw_t[96:128, 160:512])
                        if q == 8:
                            # idio part 2: feats 16384:16532 = cols 0:148
                            nc.sync.dma_start(idio_d[:, 352:500],
                                              raw_t[0:32, 0:148])

                # replicate loadings + factor vars to partition groups 1..3
                for rp in range(1, 4):
                    eng = [nc.scalar, nc.gpsimd, nc.sync][rp - 1]
                    eng.dma_start(LT[32 * rp:32 * (rp + 1), :, :],
                                  LT[0:32, :, :])
                    eng.dma_start(F4[32 * rp:32 * (rp + 1), :], F4[0:32, :])

                # Sigma: 4 m-tiles of 125 rows per sample, staged fp16, one
                # fused DMA per pair; samples rotate PE row positions so
                # LDWEIGHTS of one overlaps the matmul of the other.
                with (
                    tc.tile_pool(name="sigw", bufs=3) as sigw,
                    tc.tile_pool(name="psig", bufs=4, space="PSUM") as psigp,
                ):
                    for b0 in range(0, BL, 2):
                        st2 = sigw.tile([128, 2, 4, 500], F16, tag="sigstage")
                        gts = []
                        for bb in range(2):
                            b = b0 + bb
                            rp = b % 4
                            sp = slice(32 * rp, 32 * (rp + 1))
                            gt = sigw.tile([128, 512], BF, tag="gt",
                                           name=f"gt{bb}")
                            nc.vector.tensor_scalar_mul(
                                gt[sp, 0:500], LT[sp, b, :], F4[sp, b:b + 1])
                            gts.append((gt, sp, rp, b))
                        # half-sample psig tiles (2 m-tiles each) so evacs
                        # overlap the next matmuls instead of blocking them
                        pss = [psigp.tile([128, 1024], FP, tag="psig",
                                          name=f"psig{i}")
                               for i in range(4)]
                        for mth in range(2):
                            for bb in range(2):
                                gt, sp, rp, b = gts[bb]
                                for mtl in range(2):
                                    mt = 2 * mth + mtl
                                    mm(pss[2 * bb + mth][:125,
                                       512 * mtl:512 * mtl + 500],
                                       gt[sp, 125 * mt:125 * mt + 125],
                                       LT[sp, b, :],
                                       tp=(32 * rp, 0), start=True, stop=True)
                            for bb in range(2):
                                src = pss[2 * bb + mth].rearrange(
                                    "p (m n) -> p m n", m=2)
                                if bb % 2 == 0:
                                    nc.scalar.copy(
                                        st2[:125, bb, 2 * mth:2 * mth + 2, :],
                                        src[:125, :, 0:500])
                                else:
                                    nc.vector.tensor_copy(
                                        st2[:125, bb, 2 * mth:2 * mth + 2, :],
                                        src[:125, :, 0:500])
                        eng = [nc.sync, nc.scalar][(b0 // 2) % 2]
                        eng.dma_start(
                            sigma_d[b0:b0 + 2].rearrange(
                                "b (m p) n -> p b m n", p=125),
                            st2[:125])

    nc.compile()
    return nc


# ---------------------------------------------------------------- entry point

def kernel(**inputs):
    from concourse.bass_utils import run_bass_kernel_spmd

    prep = host_prep_shared(inputs)
    x = np.asarray(inputs["x"], np.float32)
    in_maps = []
    for core in range(NCORES):
        m = dict(prep)
        m["xT"] = host_prep_x(x[core * BL:(core + 1) * BL])
        in_maps.append(m)

    nc = build_nc()
    res = run_bass_kernel_spmd(nc, in_maps, list(range(NCORES)))
    results = res.results

    fcb_idio = np.asarray(inputs["fc_b"], np.float32)[16032:16532]
    idx = np.arange(NA)
    out = np.empty((B_FULL, NA, NA), np.float32)
    for core in range(NCORES):
        sigma = np.asarray(results[core]["sigma"]).astype(np.float32)
        idio = np.exp(np.asarray(results[core]["idio_raw"], np.float32)
                      + fcb_idio[None, :])
        sigma[:, idx, idx] += idio
        out[core * BL:(core + 1) * BL] = sigma
    return out



# revision 54
# speedup vs baseline: 1.4171x; 1.0911x over previous
"""Trainium2 Bass kernel for nn_FactorCovModel.

Model: 2-layer LSTM (H=512) over [B=256, T=64, D=500], last hidden ->
FC [512 -> 16532] -> Sigma = Lambda diag(exp(fv)) Lambda^T + diag(exp(idio)),
output [256, 500, 500].

Sharding: pure data parallel over batch, 32 samples/core on 8 cores.

Per-core device algorithm (matmul operands bf16, fp32 PSUM accumulation):
  - Weight gate axis host-permuted so PSUM col group hg holds hidden slice
    hg of ALL FOUR gates: PSUM [128 = (hg, batch), 512 = i|f|g|o x 128].
    Every ACT/DVE op is then full-128-partition and partition-aligned.
  - LSTM gates are computed column-tiled: stationary = hT chunk [128, 32],
    4 hidden-slice groups run concurrently at tile positions (0, 32j).
  - x-projection (xg0) matmuls accumulate into the same PSUM tile two
    steps ahead; recurrent matmuls then add onto it (start=False).
  - Layer-1 consumes h0T[t] directly (fused input projection, contraction
    [h0T; h1T] = 1024) plus a bias inject via a stacked-identity stationary.
  - FC runs col-packed (4 feature tiles of 512 per PSUM tile), then Lambda
    is re-laid-out via 500 PE transposes of [32, 32] blocks into
    LT [32 factors, 500 assets, 32 batch]; fvar gets exp via ACT.
  - Sigma_b = (LT_b * f_b)^T @ LT_b per sample, 4 m-tiles of 128.
  - idio raw rows go back to the host, which applies bias+exp and adds the
    diagonal (avoids diagonal APs on device).
"""

import os
import sys

sys.path.insert(0, "/opt/trn_rl_repo")

import numpy as np

import concourse.bass as bass
import concourse.mybir as mybir
from concourse import bacc
from concourse.tile import TileContext

FP = mybir.dt.float32
BF = mybir.dt.bfloat16
F16 = mybir.dt.float16
AF = mybir.ActivationFunctionType

B_FULL, T_FULL, D_IN, H = 256, 64, 500, 512
NCORES = 8
BL = B_FULL // NCORES            # 32 samples per core
NA, NF = 500, 32                 # assets, factors
OUT_DIM = NA * NF + NF + NA      # 16532
NTILE = 512                      # FC feature tile
N_FTILES = 33                    # ceil(16532/512) -> features padded to 16896
FH = N_FTILES * NTILE            # 16896
XCHUNK = 16                      # time steps per streamed xT chunk
N_PREF = 18                      # fcw feature tiles prefetched during LSTM

# gate-axis permutation: new col (hg, gate, hl) = 512*hg + 128*gate + hl maps to
# old row gate*512 + 128*hg + hl (torch gate order [i, f, g, o]).  With this
# layout, PSUM col group hg holds ALL FOUR gates of hidden slice hg along the
# free dim, so every ACT/DVE op is full-128-partition and partition-aligned.
PERM = np.array([gate * 512 + 128 * hg + hl
                 for hg in range(4) for gate in range(4) for hl in range(128)])


# ---------------------------------------------------------------- host prep

def host_prep_shared(inputs):
    w_ih0 = np.asarray(inputs["w_ih0"])[PERM]
    w_hh0 = np.asarray(inputs["w_hh0"])[PERM]
    b0 = (np.asarray(inputs["b_ih0"]) + np.asarray(inputs["b_hh0"]))[PERM]
    w_ih1 = np.asarray(inputs["w_ih1"])[PERM]
    w_hh1 = np.asarray(inputs["w_hh1"])[PERM]
    b1 = (np.asarray(inputs["b_ih1"]) + np.asarray(inputs["b_hh1"]))[PERM]
    fc_w = np.asarray(inputs["fc_w"])
    fc_b = np.asarray(inputs["fc_b"])

    w0T = np.zeros((512, 2048), np.float32)
    w0T[:500] = w_ih0.T
    w0T[500] = b0
    wh0T = np.ascontiguousarray(w_hh0.T, dtype=np.float32)
    w1xT = np.ascontiguousarray(w_ih1.T, dtype=np.float32)
    wh1T = np.ascontiguousarray(w_hh1.T, dtype=np.float32)
    b1row = np.ascontiguousarray(b1[None, :], dtype=np.float32)
    # scale the g-gate preacts by 2 so the device can use a single sigmoid
    # activation for all gates: tanh(x) = 2*sigmoid(2x) - 1
    gcols = np.zeros(2048, bool)
    for hg in range(4):
        gcols[512 * hg + 256:512 * hg + 384] = True
    for wmat in (w0T, wh0T, w1xT, wh1T, b1row):
        wmat[:, gcols] *= 2.0
    ones1 = np.ones((1, 128), np.float32)
    fcwT = np.zeros((512, FH), np.float32)
    fcwT[:, :OUT_DIM] = fc_w.T
    fcbT = np.zeros((32, 512), np.float32)
    fcbT[:, :500] = fc_b[:16000].reshape(500, 32).T
    fcbF4 = np.ascontiguousarray(
        np.tile(fc_b[16000:16032].reshape(32, 1), (4, 1)), dtype=np.float32)
    ident = np.ascontiguousarray(np.tile(np.eye(32, dtype=np.float32), (4, 1)))
    import ml_dtypes
    tobf = lambda a: np.ascontiguousarray(a, dtype=ml_dtypes.bfloat16)
    return dict(w0T=tobf(w0T), wh0T=tobf(wh0T), w1xT=tobf(w1xT),
                wh1T=tobf(wh1T), b1row=tobf(b1row), ones1=tobf(ones1),
                fcwT=tobf(fcwT), fcbT=fcbT, fcbF4=fcbF4, identt=ident,
                identb=tobf(ident))


def host_prep_x(x_core):
    """x_core [BL, T, 500] -> xT [512, T*BL], (t, b) free order, ones bias row."""
    T = x_core.shape[1]
    import ml_dtypes
    xT = np.zeros((512, T * BL), np.float32)
    xT[:500] = np.asarray(x_core, np.float32).transpose(2, 1, 0).reshape(500, T * BL)
    xT[500] = 1.0
    return np.ascontiguousarray(xT, dtype=ml_dtypes.bfloat16)


# ---------------------------------------------------------------- bass build

def build_nc(T=T_FULL):
    nc = bacc.Bacc("TRN2")

    xT_d = nc.dram_tensor("xT", [512, T * BL], BF, kind="ExternalInput")
    w0T_d = nc.dram_tensor("w0T", [512, 2048], BF, kind="ExternalInput")
    wh0T_d = nc.dram_tensor("wh0T", [512, 2048], BF, kind="ExternalInput")
    w1xT_d = nc.dram_tensor("w1xT", [512, 2048], BF, kind="ExternalInput")
    wh1T_d = nc.dram_tensor("wh1T", [512, 2048], BF, kind="ExternalInput")
    b1row_d = nc.dram_tensor("b1row", [1, 2048], BF, kind="ExternalInput")
    ones1_d = nc.dram_tensor("ones1", [1, 128], BF, kind="ExternalInput")
    fcwT_d = nc.dram_tensor("fcwT", [512, FH], BF, kind="ExternalInput")
    fcbT_d = nc.dram_tensor("fcbT", [32, 512], FP, kind="ExternalInput")
    fcbF4_d = nc.dram_tensor("fcbF4", [128, 1], FP, kind="ExternalInput")
    identt_d = nc.dram_tensor("identt", [128, 32], FP, kind="ExternalInput")
    identb_d = nc.dram_tensor("identb", [128, 32], BF, kind="ExternalInput")

    sigma_d = nc.dram_tensor("sigma", [BL, NA, NA], F16, kind="ExternalOutput")
    idio_d = nc.dram_tensor("idio_raw", [BL, NA], BF, kind="ExternalOutput")

    def mm(out, lhsT, rhs, tp, **kw):
        nc.tensor.matmul(out, lhsT, rhs,
                         tile_position=tp, skip_group_check=True, **kw)

    def tr(out, in_, identity, tp):
        nc.tensor.matmul(out, in_, identity, is_transpose=True,
                         tile_position=tp, skip_group_check=True)

    with TileContext(nc) as tc:
        with tc.tile_pool(name="persist", bufs=1) as persist:
            ones1_sb = persist.tile([1, 128], BF)
            nc.sync.dma_start(ones1_sb, ones1_d[:, :])
            b1row_sb = persist.tile([1, 2048], BF)
            nc.sync.dma_start(b1row_sb, b1row_d[:, :])
            identt_sb = persist.tile([128, 32], FP)
            nc.sync.dma_start(identt_sb, identt_d[:, :])
            identb_sb = persist.tile([128, 32], BF)
            nc.sync.dma_start(identb_sb, identb_d[:, :])
            fcbT_sb = persist.tile([32, 512], FP)
            nc.sync.dma_start(fcbT_sb, fcbT_d[:, :])
            fcbF4_sb = persist.tile([128, 1], FP)
            nc.sync.dma_start(fcbF4_sb, fcbF4_d[:, :])
            hlast = persist.tile([128, 128], BF)  # final h1T, chunk-major cols
            # prefetched fcw feature tiles (DMA issued after the LSTM weight
            # loads so it doesn't delay the LSTM start)
            fcw_pref = persist.tile([128, 4, N_PREF * 512], BF)

            # ---------------- phase 1: LSTM ----------------
            # Both layers' input projections run once per 4-step BLOCK with a
            # full [128,128] stationary (weights stream 1x per block instead
            # of per step).  Block outputs land in PSUM with rows (m, b)
            # (m = step-in-block); per-(m, j) shift-DMAs re-align them to the
            # per-step (hidden-group, b) gate layout in SBUF (xga).  The
            # per-step evac then adds xga onto the recurrent PSUM gates.
            # Layer 1 consumes blocked h0 projections SKEW steps behind l0.
            SKEW = 5
            with (
                tc.tile_pool(name="wconst", bufs=1) as wconst,
                tc.tile_pool(name="xring", bufs=2) as xring,
                tc.tile_pool(name="state", bufs=2) as state,
                tc.tile_pool(name="xgap", bufs=2) as xgap,
                tc.tile_pool(name="work", bufs=2) as work,
                tc.tile_pool(name="pgates", bufs=8, space="PSUM") as pgates,
            ):
                w0T_sb = wconst.tile([128, 4, 2048], BF)
                nc.sync.dma_start(w0T_sb, w0T_d.rearrange("(ko p) g -> p ko g", p=128))
                wh0T_sb = wconst.tile([128, 4, 2048], BF)
                nc.sync.dma_start(wh0T_sb, wh0T_d.rearrange("(ko p) g -> p ko g", p=128))
                w1x_sb = wconst.tile([128, 4, 2048], BF)
                nc.sync.dma_start(w1x_sb, w1xT_d.rearrange("(ko p) g -> p ko g", p=128))
                wh1_sb = wconst.tile([128, 4, 2048], BF)
                nc.sync.dma_start(wh1_sb, wh1T_d.rearrange("(ko p) g -> p ko g", p=128))

                xch = min(XCHUNK, T)
                n_xchunks = (T + xch - 1) // xch
                n_blocks = T // 4
                x_tiles = {}

                def load_xchunk(ci):
                    if ci >= n_xchunks:
                        return
                    xt = xring.tile([128, 4, xch * BL], BF, tag="xchunk")
                    nc.sync.dma_start(
                        xt,
                        xT_d[:, ci * xch * BL:(ci + 1) * xch * BL]
                        .rearrange("(ko p) tb -> p ko tb", p=128),
                    )
                    x_tiles[ci] = xt

                load_xchunk(0)
                # fcw prefetch rides the idle DMA bandwidth behind the weights
                nc.scalar.dma_start(
                    fcw_pref,
                    fcwT_d[:, :N_PREF * 512].rearrange("(ko p) n -> p ko n",
                                                       p=128),
                )

                def emit_xp_block(which, B, h0blk_src=None):
                    """Blocked input projection for steps [4B, 4B+4).
                    which=0: x @ w0T (bias via xT ones row);
                    which=1: h0 @ w1xT + b1 (bias via 1-row matmul).
                    Returns xga [128=(j,b), 4=m, 512] in SBUF."""
                    xpps = [pgates.tile([128, 512], FP, tag="g",
                                        name=f"xpp{which}_{j}")
                            for j in range(4)]
                    if which == 0:
                        xt = x_tiles[B // 4]
                        rel = B % 4
                        for k in range(4):
                            lhsT = xt[:, k, rel * 128:(rel + 1) * 128]
                            for j in range(4):
                                mm(xpps[j][:, :], lhsT,
                                   w0T_sb[:, k, 512 * j:512 * (j + 1)],
                                   tp=(0, 0), start=(k == 0), stop=(k == 3))
                    else:
                        for j in range(4):
                            mm(xpps[j][:, :], ones1_sb[0:1, :],
                               b1row_sb[0:1, 512 * j:512 * (j + 1)],
                               tp=(0, 0), start=True, stop=False)
                        for k in range(4):
                            lhsT = h0blk_src[:, k, :]
                            for j in range(4):
                                mm(xpps[j][:, :], lhsT,
                                   w1x_sb[:, k, 512 * j:512 * (j + 1)],
                                   tp=(0, 0), start=False, stop=(k == 3))
                    # PSUM -> SBUF block evac; the per-step partition shift
                    # happens via identity matmuls into the rec gate banks
                    xgb = xgap.tile([128, 4, 512], BF, tag=f"xgb{which}")
                    for j in range(4):
                        if j % 2 == 0:
                            nc.scalar.copy(xgb[:, j, :], xpps[j][:, :])
                        else:
                            nc.vector.tensor_copy(xgb[:, j, :], xpps[j][:, :])
                    return xgb

                def gates_banks(nm):
                    # one PSUM bank per hidden-slice col group: concurrent
                    # col-tiled matmuls into the SAME bank corrupt on HW.
                    return [pgates.tile([128, 512], FP, tag="g", name=f"{nm}{j}")
                            for j in range(4)]

                def evac(pgs, dst):
                    """Copy group j of pgs -> dst[32j:32j+32], 2 ACT / 2 DVE
                    so the gate nonlinearity isn't gated on one engine."""
                    for j in range(4):
                        s = slice(32 * j, 32 * (j + 1))
                        if j % 2 == 0:
                            nc.scalar.copy(dst[s, :], pgs[j][s, :])
                        else:
                            nc.vector.tensor_copy(dst[s, :], pgs[j][s, :])
                    return dst

                def shift_mms(pgs, xgb, m_, first):
                    """Inject the blocked input projection for step-in-block
                    m_ into the gate banks: identity matmul moves xgb rows
                    (m_, b) -> bank j rows (j, b).  Runs at PE positions
                    (32*m_, 32*j), opening each bank's accumulation group."""
                    for j in range(4):
                        mm(pgs[j][32 * j:32 * (j + 1), :],
                           identb_sb[32 * m_:32 * (m_ + 1), :],
                           xgb[32 * m_:32 * (m_ + 1), j, :],
                           tp=(32 * m_, 32 * j), start=True, stop=first)

                def gate_nonlin(ga, cprev, cnew, tag):
                    """ga [128 = (hg, b), 512 = i|f|g|o x128] SBUF -> (H, cnew).
                    g-gate preacts are host-scaled by 2, so ONE sigmoid covers
                    all four gates (tanh(x) = 2*sigmoid(2x) - 1); the fixup
                    rides DVE scalar_tensor_tensor ops."""
                    a = work.tile([128, 512], FP, tag=f"act_{tag}")
                    nc.scalar.activation(a, ga, AF.Sigmoid)
                    # t1h = (sig(2g) - 0.5) * i = (i * tanh(g)) / 2
                    t1h = work.tile([128, 128], FP, tag=f"t1_{tag}")
                    nc.vector.scalar_tensor_tensor(
                        t1h, a[:, 256:384], 0.5, a[:, 0:128],
                        mybir.AluOpType.subtract, mybir.AluOpType.mult)
                    cn = cnew
                    if cprev is None:
                        nc.vector.tensor_scalar_mul(cn, t1h, 2.0)
                    else:
                        t2 = work.tile([128, 128], FP, tag=f"t2_{tag}")
                        nc.vector.tensor_mul(t2, a[:, 128:256], cprev)
                        nc.vector.scalar_tensor_tensor(
                            cn, t1h, 2.0, t2,
                            mybir.AluOpType.mult, mybir.AluOpType.add)
                    tcn = work.tile([128, 128], FP, tag=f"tc_{tag}")
                    nc.scalar.activation(tcn, cn, AF.Tanh)
                    hh = work.tile([128, 128], FP, tag=f"h_{tag}")
                    nc.vector.tensor_mul(hh, a[:, 384:512], tcn)
                    return hh, cn

                def transpose_cols(hh, dst_cols, tag):
                    """hh [128=(hg,b),128] -> bf16 transposed into dst_cols[k]
                    [128, 32] slices; each 32-col transpose gets its OWN psum
                    bank (concurrent row-tiled transposes corrupt on HW)."""
                    for k in range(4):
                        ptk = pgates.tile([128, 32], FP, tag="g",
                                          name=f"pt{tag}{k}")
                        tr(ptk, hh[32 * k:32 * (k + 1), :],
                           identt_sb[32 * k:32 * (k + 1), :], (32 * k, 0))
                        nc.vector.tensor_copy(dst_cols[k], ptk)

                c0 = c1 = None
                h1T = None
                h0blk_cur = h0blk_prev = None
                xga0 = {}
                xga1 = {}
                pend0 = pend1 = None  # deferred transposes (h, dst_cols[, ht])
                xga0[0] = emit_xp_block(0, 0)
                for s in range(T + SKEW):
                    # ---- all matmuls first: the PE queue is in-order, so
                    # transposes are DEFERRED one superstep — by emission
                    # time their nonlinearity inputs are long done, so the
                    # PE never idles waiting on ACT/DVE.
                    if pend0 is not None:
                        transpose_cols(pend0[0], pend0[1], "l0")
                        pend0 = None
                    if s % 4 == 0 and 4 <= s <= T:
                        # xp1 block for the just-finished l0 block
                        bprev = s // 4 - 1
                        xga1[bprev] = emit_xp_block(1, bprev, h0blk_cur)
                    if s < T:
                        m, B = s % 4, s // 4
                        if m == 0:
                            h0blk_prev = h0blk_cur
                            h0blk_cur = state.tile([128, 4, 128], BF,
                                                   tag="h0blk")
                            if B % 4 == 2:
                                load_xchunk(B // 4 + 1)
                        pgs0 = gates_banks("g0_")
                        shift_mms(pgs0, xga0[B], m, first=(s == 0))
                        if s > 0:
                            src = h0blk_cur if m > 0 else h0blk_prev
                            mp = (s - 1) % 4
                            for k in range(4):
                                lhsT = src[:, k, 32 * mp:32 * (mp + 1)]
                                for j in range(4):
                                    mm(pgs0[j][32 * j:32 * (j + 1), :], lhsT,
                                       wh0T_sb[:, k, 512 * j:512 * (j + 1)],
                                       tp=(0, 32 * j),
                                       start=False, stop=(k == 3))
                    if s >= SKEW:
                        if pend1 is not None:
                            transpose_cols(pend1[0], pend1[1], "l1")
                            h1T = pend1[2]
                            pend1 = None
                        t1_ = s - SKEW
                        m1, B1 = t1_ % 4, t1_ // 4
                        pgs1 = gates_banks("g1_")
                        shift_mms(pgs1, xga1[B1], m1, first=(t1_ == 0))
                        if t1_ > 0:
                            for k in range(4):
                                lhsT = h1T[:, 32 * k:32 * (k + 1)]
                                for j in range(4):
                                    mm(pgs1[j][32 * j:32 * (j + 1), :], lhsT,
                                       wh1_sb[:, k, 512 * j:512 * (j + 1)],
                                       tp=(0, 32 * j),
                                       start=False, stop=(k == 3))

                    # ---- xp0 block for an upcoming block
                    if s < T and m == 1 and B + 1 < n_blocks:
                        xga0[B + 1] = emit_xp_block(0, B + 1)

                    # ---- layer-0 post-processing
                    if s < T:
                        a0 = work.tile([128, 512], FP, tag="ga_l0")
                        evac(pgs0, a0)
                        c0n = state.tile([128, 128], FP, tag="c0")
                        h0, c0 = gate_nonlin(a0, c0, c0n, "l0")
                        pend0 = (h0, [h0blk_cur[:, k, 32 * m:32 * (m + 1)]
                                      for k in range(4)])

                    # ---- layer-1 post-processing
                    if s >= SKEW:
                        a1 = work.tile([128, 512], FP, tag="ga_l1")
                        evac(pgs1, a1)
                        c1n = state.tile([128, 128], FP, tag="c1")
                        h1, c1 = gate_nonlin(a1, c1, c1n, "l1")
                        h1T_n = state.tile([128, 128], BF, tag="ht_l1")
                        pend1 = (h1, [h1T_n[:, 32 * k:32 * (k + 1)]
                                      for k in range(4)], h1T_n)

                transpose_cols(pend1[0], pend1[1], "l1")
                nc.vector.tensor_copy(hlast, pend1[2])

            # ---------------- phase 2: FC + Lambda layout + Sigma ----------------
            # LT holds loadings as [factor, b, asset] replicated on 4
            # partition groups after FC, so sigma matmuls can rotate PE row
            # positions (hides LDWEIGHTS) and stream contiguous rows.
            with tc.tile_pool(name="lt", bufs=1) as ltp:
                LT = ltp.tile([128, 32, 500], BF)   # [factor(x4), b, asset]
                F4 = ltp.tile([128, 32], FP)        # exp(fvar+bias) x4 groups

                with (
                    tc.tile_pool(name="fcw", bufs=N_FTILES - N_PREF) as fcwp,
                    tc.tile_pool(name="rawp", bufs=3) as rawp,
                    tc.tile_pool(name="pfc", bufs=4, space="PSUM") as pfcp,
                    tc.tile_pool(name="plt", bufs=4, space="PSUM") as pltp,
                ):
                    # bulk-load the non-prefetched fcw tiles across all three
                    # DMA queues (one queue moves only ~50 GB/s)
                    fcw_rest = {}
                    for idx, jj in enumerate(range(N_PREF, N_FTILES)):
                        fcw_t = fcwp.tile([128, 4, 512], BF, tag="fcw",
                                          name=f"fcwr{jj}")
                        eng = [nc.sync, nc.scalar, nc.gpsimd][idx % 3]
                        eng.dma_start(
                            fcw_t,
                            fcwT_d[:, jj * 512:(jj + 1) * 512]
                            .rearrange("(ko p) n -> p ko n", p=128),
                        )
                        fcw_rest[jj] = fcw_t

                    n_quads = (N_FTILES + 3) // 4      # 9 (last quad: 1 tile)
                    for q in range(n_quads):
                        rr = range(4) if q < 8 else range(1)
                        raw_t = rawp.tile([128, 512], BF, tag="raw")
                        srcs, pfcs = {}, {}
                        for r in rr:
                            jj = 4 * q + r
                            if jj < N_PREF:
                                srcs[r] = fcw_pref[:, :, jj * 512:(jj + 1) * 512]
                            else:
                                srcs[r] = fcw_rest[jj]
                            pfcs[r] = pfcp.tile([128, 512], FP, tag="pfc",
                                                name=f"pfc{r}")
                        # k outer / r inner: consecutive matmuls sit at
                        # different PE col positions so LDWEIGHTS overlaps
                        for k in range(4):
                            for r in rr:
                                mm(pfcs[r][32 * r:32 * (r + 1), :],
                                   hlast[:, 32 * k:32 * (k + 1)],
                                   srcs[r][:, k, :],
                                   tp=(0, 32 * r),
                                   start=(k == 0), stop=(k == 3))
                        for r in rr:
                            s = slice(32 * r, 32 * (r + 1))
                            if r % 2 == 0:
                                nc.scalar.copy(raw_t[s, :], pfcs[r][s, :])
                            else:
                                nc.vector.tensor_copy(raw_t[s, :], pfcs[r][s, :])

                        # Lambda blocks -> transpose into LT.  g outer / r
                        # inner so consecutive transposes rotate PE row
                        # positions (hides the per-transpose weight load).
                        for g in range(0, 16, 4):
                            pts = {}
                            for r in rr:
                                jj = 4 * q + r
                                nblk = 16 if jj < 31 else (4 if jj == 31 else 0)
                                if g < nblk:
                                    pts[r] = pltp.tile([32, 128], BF,
                                                       tag="plt",
                                                       name=f"plt{r}")
                            for a in range(4):
                                for r, pt in pts.items():
                                    blk = g + a
                                    tr(pt[:, 32 * a:32 * (a + 1)],
                                       raw_t[32 * r:32 * (r + 1),
                                             32 * blk:32 * (blk + 1)],
                                       identb_sb[32 * r:32 * (r + 1), :],
                                       (32 * r, 0))
                            for r, pt in pts.items():
                                a0 = ((4 * q + r) * 512 + 32 * g) // 32
                                nc.vector.tensor_tensor(
                                    LT[0:32, :, a0:a0 + 4],
                                    pt.rearrange("f (a b) -> f b a", a=4),
                                    fcbT_sb[:, None, a0:a0 + 4]
                                    .to_broadcast([32, 32, 4]),
                                    mybir.AluOpType.add,
                                )
                        if q == 7:
                            # fvar feats 16000:16032 = raw cols 128:160 (r==3)
                            ptf_full = pltp.tile([32, 128], BF, tag="plt",
                                                 name="pltf")
                            ptf = ptf_full[:, 0:32]
                            tr(ptf, raw_t[96:128, 128:160],
                               identb_sb[96:128, :], (96, 0))
                            nc.scalar.activation(F4[0:32, :], ptf, AF.Exp,
                                                 bias=fcbF4_sb[0:32, 0:1])
                            # idio part 1: feats 16032:16384 = cols 160:512
                            nc.sync.dma_start(idio_d[:, 0:352],
                                              raw_t[96:128, 160:512])
                        if q == 8:
                            # idio part 2: feats 16384:16532 = cols 0:148
                            nc.sync.dma_start(idio_d[:, 352:500],
                                              raw_t[0:32, 0:148])

                # replicate loadings + factor vars to partition groups 1..3
                for rp in range(1, 4):
                    eng = [nc.scalar, nc.gpsimd, nc.sync][rp - 1]
                    eng.dma_start(LT[32 * rp:32 * (rp + 1), :, :],
                                  LT[0:32, :, :])
                    eng.dma_start(F4[32 * rp:32 * (rp + 1), :], F4[0:32, :])

                # Sigma: 4 m-tiles of 125 rows per sample, staged fp16, one
                # fused DMA per pair; samples rotate PE row positions so
                # LDWEIGHTS of one overlaps the matmul of the other.
                with (
                    tc.tile_pool(name="sigw", bufs=3) as sigw,
                    tc.tile_pool(name="psig", bufs=8, space="PSUM") as psigp,
                ):
                    for b0 in range(0, BL, 2):
                        st2 = sigw.tile([128, 2, 4, 500], F16, tag="sigstage")
                        gts = []
                        for bb in range(2):
                            b = b0 + bb
                            rp = b % 4
                            sp = slice(32 * rp, 32 * (rp + 1))
                            gt = sigw.tile([128, 512], BF, tag="gt",
                                           name=f"gt{bb}")
                            nc.vector.tensor_scalar_mul(
                                gt[sp, 0:500], LT[sp, b, :], F4[sp, b:b + 1])
                            gts.append((gt, sp, rp, b))
                        for mt in range(4):
                            pp = []
                            for bb in range(2):
                                gt, sp, rp, b = gts[bb]
                                ps = psigp.tile([128, 512], FP, tag="psig",
                                                name=f"ps{bb}")
                                mm(ps[:125, 0:500],
                                   gt[sp, 125 * mt:125 * mt + 125],
                                   LT[sp, b, :],
                                   tp=(32 * rp, 0), start=True, stop=True)
                                pp.append(ps)
                            for bb in range(2):
                                if bb == 0:
                                    nc.scalar.copy(st2[:125, bb, mt, :],
                                                   pp[bb][:125, 0:500])
                                else:
                                    nc.vector.tensor_copy(st2[:125, bb, mt, :],
                                                          pp[bb][:125, 0:500])
                        eng = [nc.sync, nc.scalar][(b0 // 2) % 2]
                        eng.dma_start(
                            sigma_d[b0:b0 + 2].rearrange(
                                "b (m p) n -> p b m n", p=125),
                            st2[:125])

    nc.compile()
    return nc


# ---------------------------------------------------------------- entry point

def kernel(**inputs):
    from concourse.bass_utils import run_bass_kernel_spmd

    prep = host_prep_shared(inputs)
    x = np.asarray(inputs["x"], np.float32)
    in_maps = []
    for core in range(NCORES):
        m = dict(prep)
        m["xT"] = host_prep_x(x[core * BL:(core + 1) * BL])
        in_maps.append(m)

    nc = build_nc()
    res = run_bass_kernel_spmd(nc, in_maps, list(range(NCORES)))
    results = res.results

    fcb_idio = np.asarray(inputs["fc_b"], np.float32)[16032:16532]
    idx = np.arange(NA)
    out = np.empty((B_FULL, NA, NA), np.float32)
    for core in range(NCORES):
        sigma = np.asarray(results[core]["sigma"]).astype(np.float32)
        idio = np.exp(np.asarray(results[core]["idio_raw"], np.float32)
                      + fcb_idio[None, :])
        sigma[:, idx, idx] += idio
        out[core * BL:(core + 1) * BL] = sigma
    return out

